# revision 1
# baseline (speedup 1.0000x reference)
"""Trainium2 Bass kernel for nn_AttentionModule (Bahdanau-style attention).

Reference computation (S=512, B=64, H=1024, F=2H):
    cat    = concat([hidden bcast to (S,B,H), encoder_states], -1)      [S,B,2H]
    scores = tanh(cat @ W_attn.T + b_attn) @ W_attn2.T + b_attn2        [S,B,1]
    attn   = softmax(scores[..., 0].T, axis=-1)                         [B,S]
    applied= einsum("bs,sbh->bh", attn, encoder_states)                 [B,H]
    out    = tanh(concat([decoder_out, applied], -1) @ W_comb.T + b_comb)

Sharding: data-parallel over B across 8 cores (8 batch rows per core).
All heavy math stays on-device; the host only slices, transposes and casts
the per-core shards.

Per-core structure:
  - enc_t [8, 1024, 512] bf16: encoder slice with H on partitions. One 2MB
    DMA per batch row (3D access pattern).
  - Main matmul per (b, ft): T^T[f, s] = sum_h W2T[h, f] * encT[h, s] with
    the weight chunk stationary, bf16 at full PE rate, fp32 PSUM.
  - tanh fused on ScalarE with per-partition bias b_attn[f] + hid_part[b, f]
    (hid_part computed on device in a preamble).
  - scores via PE matmul contracting f: lhsT = W_attn2 replicated to 8 cols
    (all psum rows identical -> row b used directly, no partition shifts).
  - softmax over s on 8 partitions (reduce_max(negate) -> Exp with bias and
    fused accumulate -> reciprocal -> scale).
  - attention row broadcast across partitions via a DRAM bounce DMA.
  - applied^T[h, b] on VectorE: multiply resident encT tiles by the broadcast
    attention row, reduce along s. Written column-wise into appT (fp32 output)
    and converted per-b to bf16 for the final matmul.
  - Final combine matmul (bf16) with biases folded as K=1 matmul terms.

Known pitfalls baked into this implementation:
  - bf16 input arrays with tiny rows (16B) get corrupted on the host->device
    path, so every small tensor ships as fp32 and is cast on device.
  - fp32 matmuls run at 1/4 rate; fp32r is full rate but only PE/DMA may
    touch f32r-typed tensors; bf16 everywhere is simplest at full rate.
  - 16/32-bit matmul operand mixing is rejected by the compiler.
  - Multi-dim rearrange DMAs with 16-byte inner blocks corrupt data on HW;
    only used with >=1KB inner blocks here (the encoder load).
"""

import numpy as np

S, B, H = 512, 64, 1024
F = 2 * H
NCORES = 8
BL = B // NCORES          # 8 batch rows per core
KH = H // 128             # 8 contraction chunks over H
KF = F // 128             # 16 feature tiles

_CACHE = {}


def _build(num_devices=NCORES):
    from contextlib import ExitStack

    import concourse.tile as tile
    from concourse import bacc, mybir
    from concourse.masks import make_identity

    f32 = mybir.dt.float32
    bf16 = mybir.dt.bfloat16
    AF = mybir.ActivationFunctionType
    ALU = mybir.AluOpType
    AX = mybir.AxisListType

    nc = bacc.Bacc("TRN2", target_bir_lowering=False, debug=False,
                   num_devices=num_devices)

    enc_t = nc.dram_tensor("enc_t", [BL, H, S], bf16, kind="ExternalInput").ap()
    wat = nc.dram_tensor("wat", [F, F], bf16, kind="ExternalInput").ap()
    wct = nc.dram_tensor("wct", [F, H], bf16, kind="ExternalInput").ap()
    hidT = nc.dram_tensor("hidT", [H, BL], f32, kind="ExternalInput").ap()
    decT = nc.dram_tensor("decT", [H, BL], f32, kind="ExternalInput").ap()
    w2rep = nc.dram_tensor("w2rep", [F, BL], f32, kind="ExternalInput").ap()
    b_attn_d = nc.dram_tensor("b_attn", [1, F], f32, kind="ExternalInput").ap()
    b_comb_d = nc.dram_tensor("b_comb", [1, H], f32, kind="ExternalInput").ap()
    out_d = nc.dram_tensor("out", [BL, H], f32, kind="ExternalOutput").ap()
    appT_d = nc.dram_tensor("appliedT", [H, BL], f32,
                            kind="ExternalOutput").ap()

    with tile.TileContext(nc) as tc:
        with ExitStack() as ctx:
            consts = ctx.enter_context(tc.tile_pool(name="consts", bufs=1))
            enct_pool = ctx.enter_context(tc.tile_pool(name="enct", bufs=2))
            w1_pool = ctx.enter_context(tc.tile_pool(name="w1t", bufs=2))
            tanh_pool = ctx.enter_context(tc.tile_pool(name="tanh", bufs=18))
            attn_pool = ctx.enter_context(tc.tile_pool(name="attn", bufs=2))
            abc_pool = ctx.enter_context(tc.tile_pool(name="abc", bufs=2))
            dram_pool = ctx.enter_context(
                tc.tile_pool(name="dram", bufs=2, space="DRAM"))
            scr_pool = ctx.enter_context(tc.tile_pool(name="scr", bufs=2))
            small_pool = ctx.enter_context(tc.tile_pool(name="small", bufs=4))
            wct_pool = ctx.enter_context(tc.tile_pool(name="wct", bufs=4))
            psT_pool = ctx.enter_context(
                tc.tile_pool(name="psT", bufs=2, space="PSUM"))
            psSc_pool = ctx.enter_context(
                tc.tile_pool(name="psSc", bufs=2, space="PSUM"))
            psPre_pool = ctx.enter_context(
                tc.tile_pool(name="psPre", bufs=2, space="PSUM"))
            psOut_pool = ctx.enter_context(
                tc.tile_pool(name="psOut", bufs=2, space="PSUM"))

            # ---- encoder prefetch for b=0 (emitted first so its DMA leads) --
            def load_enct(b):
                t = enct_pool.tile([128, KH * S], bf16, tag="enct",
                                   name="enct")
                nc.sync.dma_start(
                    t.rearrange("p (k s) -> p k s", s=S),
                    enc_t[b].rearrange("(k p) s -> p k s", p=128))
                return t

            enct_tiles = {0: load_enct(0)}

            # ---- W2T chunk 0 early so the first main matmul can start ----
            w2t_sb = consts.tile([128, KH * F], bf16)
            nc.sync.dma_start(w2t_sb[:, 0:F], wat[H:H + 128, :])

            # ---- small constants (shipped fp32, cast on device) ----
            identity = consts.tile([128, 128], f32)
            make_identity(nc, identity[:])
            ones_bf = consts.tile([1, BL], bf16)
            nc.vector.memset(ones_bf[:], 1.0)
            b_attn_32 = consts.tile([1, F], f32)
            nc.sync.dma_start(b_attn_32[:], b_attn_d[:])
            b_attn_sb = consts.tile([1, F], bf16)
            nc.vector.tensor_copy(b_attn_sb[:], b_attn_32[:])
            b_comb_32 = consts.tile([1, H], f32)
            nc.sync.dma_start(b_comb_32[:], b_comb_d[:])
            b_comb_sb = consts.tile([1, H], bf16)
            nc.vector.tensor_copy(b_comb_sb[:], b_comb_32[:])

            hidT_32 = consts.tile([128, KH * BL], f32)
            decT_32 = consts.tile([128, KH * BL], f32)
            w2rep_32 = consts.tile([128, KF * BL], f32)
            for kc in range(KH):
                nc.sync.dma_start(hidT_32[:, kc * BL:(kc + 1) * BL],
                                  hidT[kc * 128:(kc + 1) * 128, :])
                nc.sync.dma_start(decT_32[:, kc * BL:(kc + 1) * BL],
                                  decT[kc * 128:(kc + 1) * 128, :])
            for ft in range(KF):
                nc.sync.dma_start(w2rep_32[:, ft * BL:(ft + 1) * BL],
                                  w2rep[ft * 128:(ft + 1) * 128, :])
            hidT_sb = consts.tile([128, KH * BL], bf16)
            nc.vector.tensor_copy(hidT_sb[:], hidT_32[:])
            decT_sb = consts.tile([128, KH * BL], bf16)
            nc.vector.tensor_copy(decT_sb[:], decT_32[:])
            w2rep_sb = consts.tile([128, KF * BL], bf16)
            nc.vector.tensor_copy(w2rep_sb[:], w2rep_32[:])

            hidbT_sb = consts.tile([128, KF * BL], f32)
            appT_sb = consts.tile([128, KH * BL], f32)
            appT_bf = consts.tile([128, KH * BL], bf16)

            # ---- hid_part preamble: hidb[b, f] = hidden @ W1.T + b_attn ----
            hidb_row = consts.tile([BL, F], f32)
            for fc in range(F // 512):
                ph = psPre_pool.tile([BL, 512], f32, tag="pre", name=f"ph{fc}")
                for kc in range(KH):
                    w1c = w1_pool.tile([128, 512], bf16, tag="w1t", name="w1c")
                    nc.sync.dma_start(
                        w1c[:], wat[kc * 128:(kc + 1) * 128,
                                    fc * 512:(fc + 1) * 512])
                    nc.tensor.matmul(
                        ph[:], hidT_sb[:, kc * BL:(kc + 1) * BL], w1c[:],
                        start=(kc == 0), stop=False)
                nc.tensor.matmul(
                    ph[:], ones_bf[:], b_attn_sb[:, fc * 512:(fc + 1) * 512],
                    start=False, stop=True)
                nc.vector.tensor_copy(hidb_row[:, fc * 512:(fc + 1) * 512],
                                      ph[:])
            # transpose [8, 2048] -> hidbT_sb [128, KF*8] (f on partitions)
            for ft in range(KF):
                ptp = psPre_pool.tile([128, BL], f32, tag="pre", name="ptp")
                nc.tensor.transpose(ptp[:],
                                    hidb_row[:, ft * 128:(ft + 1) * 128],
                                    identity[:BL, :BL])
                nc.vector.tensor_copy(hidbT_sb[:, ft * BL:(ft + 1) * BL],
                                      ptp[:])

            # ---- remaining W2T chunks ----
            for kc in range(1, KH):
                nc.sync.dma_start(
                    w2t_sb[:, kc * F:(kc + 1) * F],
                    wat[H + kc * 128: H + (kc + 1) * 128, :])

            # ---- main loop over local batch rows ----
            for b in range(BL):
                if b + 1 < BL:
                    enct_tiles[b + 1] = load_enct(b + 1)
                et = enct_tiles.pop(b)

                def etk(kc):
                    return et[:, kc * S:(kc + 1) * S]

                th = []
                for ft in range(KF):
                    pT = psT_pool.tile([128, S], f32, tag="pT", name="pT")
                    for kc in range(KH):
                        nc.tensor.matmul(
                            pT[:],
                            w2t_sb[:, kc * F + ft * 128:
                                   kc * F + (ft + 1) * 128],
                            etk(kc),
                            start=(kc == 0), stop=(kc == KH - 1))
                    t = tanh_pool.tile([128, S], bf16, tag="tanh", name="tanh")
                    nc.scalar.activation(
                        t[:], pT[:], AF.Tanh,
                        bias=hidbT_sb[:, ft * BL + b: ft * BL + b + 1],
                        scale=1.0)
                    th.append(t)

                psc = psSc_pool.tile([BL, S], f32, tag="psc", name="psc")
                for ft in range(KF):
                    nc.tensor.matmul(
                        psc[:],
                        w2rep_sb[:, ft * BL:(ft + 1) * BL],
                        th[ft][:],
                        start=(ft == 0), stop=(ft == KF - 1))

                negmax = small_pool.tile([BL, 1], f32, tag="negmax",
                                         name="negmax")
                nc.vector.reduce_max(negmax[:], psc[:], axis=AX.X, negate=True)
                attn = attn_pool.tile([BL, S], bf16, tag="attn", name="attn")
                sumexp = small_pool.tile([BL, 1], f32, tag="sumexp",
                                         name="sumexp")
                nc.scalar.activation(attn[:], psc[:], AF.Exp,
                                     bias=negmax[:], scale=1.0,
                                     accum_out=sumexp[:])
                recip = small_pool.tile([BL, 1], f32, tag="recip", name="recip")
                nc.vector.reciprocal(recip[:], sumexp[:])
                nc.vector.tensor_scalar_mul(attn[:], attn[:], recip[:])

                # broadcast attn row across 128 partitions via DRAM bounce
                attn_dr = dram_pool.tile([1, S], bf16, tag="attn_dr",
                                         name="attn_dr")
                nc.sync.dma_start(attn_dr[:], attn[0:1, :])
                abc = abc_pool.tile([128, S], bf16, tag="abc", name="abc")
                nc.sync.dma_start(abc[:],
                                  attn_dr[0:1, :].to_broadcast((128, S)))

                for kc in range(KH):
                    scr = scr_pool.tile([128, S], f32, tag="scr", name="scr")
                    nc.vector.tensor_tensor(out=scr[:], in0=etk(kc),
                                            in1=abc[:], op=ALU.mult)
                    nc.vector.reduce_sum(
                        appT_sb[:, kc * BL + b: kc * BL + b + 1],
                        scr[:], axis=AX.X)
                nc.vector.tensor_copy(
                    appT_bf.rearrange("p (k b) -> p k b", b=BL)[:, :, b],
                    appT_sb.rearrange("p (k b) -> p k b", b=BL)[:, :, b])

            # ---- final combine: out = tanh([dec | applied] @ Wc.T + b_comb) --
            pouts = [psOut_pool.tile([BL, 512], f32, tag="pout", name=f"po{i}")
                     for i in range(2)]
            for kc in range(2 * KH):
                if kc < KH:
                    lhs = decT_sb[:, kc * BL:(kc + 1) * BL]
                else:
                    lhs = appT_bf[:, (kc - KH) * BL:(kc - KH + 1) * BL]
                w = wct_pool.tile([128, H], bf16, tag="wct", name="wctt")
                nc.sync.dma_start(w[:], wct[kc * 128:(kc + 1) * 128, :])
                for fc in range(2):
                    nc.tensor.matmul(
                        pouts[fc][:], lhs, w[:, fc * 512:(fc + 1) * 512],
                        start=(kc == 0), stop=False)
            for fc in range(2):
                nc.tensor.matmul(
                    pouts[fc][:], ones_bf[:],
                    b_comb_sb[:, fc * 512:(fc + 1) * 512],
                    start=False, stop=True)

            out_sb = consts.tile([BL, H], f32)
            for fc in range(2):
                nc.scalar.activation(out_sb[:, fc * 512:(fc + 1) * 512],
                                     pouts[fc][:], AF.Tanh)
            nc.sync.dma_start(out_d[:], out_sb[:])
            for kc in range(KH):
                nc.sync.dma_start(appT_d[kc * 128:(kc + 1) * 128, :],
                                  appT_sb[:, kc * BL:(kc + 1) * BL])

    nc.compile()
    return nc


def _get_nc():
    if "nc" not in _CACHE:
        _CACHE["nc"] = _build()
    return _CACHE["nc"]


def make_in_maps(inputs):
    import ml_dtypes
    bf = ml_dtypes.bfloat16

    inp = {k: np.asarray(v, dtype=np.float32) for k, v in inputs.items()}
    hidden = inp["hidden"]
    decoder_out = inp["decoder_out"]
    encoder_states = inp["encoder_states"]
    W_attn = inp["W_attn"]
    b_attn = inp["b_attn"]
    W_attn2 = inp["W_attn2"]
    W_comb = inp["W_comb"]
    b_comb = inp["b_comb"]
    # b_attn2 shifts every score equally -> softmax-invariant, unused.

    wat = np.ascontiguousarray(W_attn.T).astype(bf)
    wct = np.ascontiguousarray(W_comb.T).astype(bf)
    w2rep = np.ascontiguousarray(np.repeat(W_attn2.reshape(F, 1), BL, axis=1))
    b_attn_2d = np.ascontiguousarray(b_attn.reshape(1, F))
    b_comb_2d = np.ascontiguousarray(b_comb.reshape(1, H))

    in_maps = []
    for c in range(NCORES):
        sl = slice(c * BL, (c + 1) * BL)
        in_maps.append({
            "enc_t": np.ascontiguousarray(
                encoder_states[:, sl, :].transpose(1, 2, 0)).astype(bf),
            "wat": wat,
            "wct": wct,
            "hidT": np.ascontiguousarray(hidden[sl].T),
            "decT": np.ascontiguousarray(decoder_out[sl].T),
            "w2rep": w2rep,
            "b_attn": b_attn_2d,
            "b_comb": b_comb_2d,
        })
    return in_maps


def kernel(**inputs):
    from concourse.bass_utils import run_bass_kernel_spmd

    in_maps = make_in_maps(inputs)
    nc = _get_nc()
    res = run_bass_kernel_spmd(nc, in_maps, list(range(NCORES)))
    out = np.concatenate([res.results[c]["out"] for c in range(NCORES)], axis=0)
    applied = np.concatenate(
        [np.ascontiguousarray(res.results[c]["appliedT"].T)
         for c in range(NCORES)], axis=0)
    return out.astype(np.float32), applied.astype(np.float32)



# revision 15
# speedup vs baseline: 1.3297x; 1.3297x over previous
"""Trainium2 Bass kernel for nn_AttentionModule (Bahdanau-style attention).

Reference computation (S=512, B=64, H=1024, F=2H):
    cat    = concat([hidden bcast to (S,B,H), encoder_states], -1)      [S,B,2H]
    scores = tanh(cat @ W_attn.T + b_attn) @ W_attn2.T + b_attn2        [S,B,1]
    attn   = softmax(scores[..., 0].T, axis=-1)                         [B,S]
    applied= einsum("bs,sbh->bh", attn, encoder_states)                 [B,H]
    out    = tanh(concat([decoder_out, applied], -1) @ W_comb.T + b_comb)

Sharding: data-parallel over B across 8 cores (8 batch rows per core).

Optimized structure (vs the bf16 baseline):
  - The dominant matmul T[f,s] = W1e @ enc (per b: [2048,1024]@[1024,512])
    runs in fp8e4 with perf_mode=DoubleRow: each instruction consumes two
    128-deep contraction chunks at once (~1.8x the bf16 rate).  W1e ships
    from the host pre-scaled by 256 (fp8e4 subnormal range) and the tanh
    activation un-scales with scale=1/256.  enc ships bf16 (needed at
    bf16 precision for the attention application) and is cast to fp8 on
    GpSimd, which is otherwise idle.
  - Batch rows are processed in two groups of 4 with b innermost so each
    DoubleRow weight load is reused by 4 matmuls (LDWEIGHTS has no FWL in
    DoubleRow mode, so un-reused loads cost ~256 PE cycles each).
  - The scores matmul (contract f: w2 . tanh) accumulates inline, one
    partial matmul per (ft,b) emitted one ft behind the main sweep, into
    per-b PSUM slices packed 4-to-a-bank at partition offsets 0/32/64/96.
  - hid @ W1h.T preamble and the final combine run operand-swapped
    (weights stationary, batch as the 8-wide moving operand) producing
    transposed outputs directly -- no PE transposes, and the decoder_out
    half of the final combine is emitted early, off the critical tail.
  - softmax / attention-apply (VectorE mult+reduce over resident bf16
    enc tiles) unchanged from baseline, overlapping the next group's PE
    sweep; attention row broadcast via DRAM bounce.

Known pitfalls baked in (from the baseline, still honored):
  - small tensors ship fp32 and are cast on device (tiny bf16 rows get
    corrupted host->device).
  - multi-dim rearrange DMAs only with >=1KB inner blocks (encoder/w2t).
  - 16/32-bit matmul operand mixing rejected; fp8 pairs must both be fp8.
"""

import numpy as np

S, B, H = 512, 64, 1024
F = 2 * H
NCORES = 8
BL = B // NCORES          # 8 batch rows per core
KH = H // 128             # 8 contraction chunks over H
KF = F // 128             # 16 feature tiles
GB = 2                    # batch rows per group (PSUM-bank limited: per-b
                          # score slices pack at partition offsets 0/32 only;
                          # quadrant 3 (offset 96) is unusable on TRN2)
NG = BL // GB             # 2 groups
WSCALE = 256.0            # fp8 weight pre-scale (power of 2)

_CACHE = {}


def _build(num_devices=NCORES):
    from contextlib import ExitStack

    import concourse.tile as tile
    from concourse import bacc, mybir

    f32 = mybir.dt.float32
    bf16 = mybir.dt.bfloat16
    fp8 = mybir.dt.float8e4
    AF = mybir.ActivationFunctionType
    ALU = mybir.AluOpType
    AX = mybir.AxisListType
    DR = mybir.MatmulPerfMode.DoubleRow

    nc = bacc.Bacc("TRN2", target_bir_lowering=False, debug=False,
                   num_devices=num_devices)

    enc_t = nc.dram_tensor("enc_t", [BL, H, S], bf16, kind="ExternalInput").ap()
    w1h = nc.dram_tensor("w1h", [H, F], bf16, kind="ExternalInput").ap()
    w2t = nc.dram_tensor("w2t", [H, F], fp8, kind="ExternalInput").ap()
    wct = nc.dram_tensor("wct", [F, H], bf16, kind="ExternalInput").ap()
    hidT = nc.dram_tensor("hidT", [H, BL], f32, kind="ExternalInput").ap()
    decT = nc.dram_tensor("decT", [H, BL], f32, kind="ExternalInput").ap()
    w2rep = nc.dram_tensor("w2rep", [F, BL], f32, kind="ExternalInput").ap()
    b_attn_d = nc.dram_tensor("b_attn", [1, F], f32, kind="ExternalInput").ap()
    b_comb_d = nc.dram_tensor("b_comb", [1, H], f32, kind="ExternalInput").ap()
    outT_d = nc.dram_tensor("outT", [H, BL], f32, kind="ExternalOutput").ap()
    appT_d = nc.dram_tensor("appliedT", [H, BL], f32,
                            kind="ExternalOutput").ap()

    with tile.TileContext(nc) as tc:
        with ExitStack() as ctx:
            consts = ctx.enter_context(tc.tile_pool(name="consts", bufs=1))
            encbf_pool = ctx.enter_context(tc.tile_pool(name="encbf", bufs=BL))
            encq_pool = ctx.enter_context(tc.tile_pool(name="encq", bufs=BL))
            w1h_pool = ctx.enter_context(tc.tile_pool(name="w1h", bufs=2))
            wct_pool = ctx.enter_context(tc.tile_pool(name="wct", bufs=4))
            th_pool = ctx.enter_context(tc.tile_pool(name="th", bufs=10))
            attn_pool = ctx.enter_context(tc.tile_pool(name="attn", bufs=2))
            abc_pool = ctx.enter_context(tc.tile_pool(name="abc", bufs=2))
            scr_pool = ctx.enter_context(tc.tile_pool(name="scr", bufs=2))
            small_pool = ctx.enter_context(tc.tile_pool(name="small", bufs=8))
            dram_pool = ctx.enter_context(
                tc.tile_pool(name="dram", bufs=2, space="DRAM"))
            # PSUM budget (8 banks): psPre(1) closes before the main sweep,
            # then psOut(1) + psT(4) + psSc(3, one [8,512] bank per batch
            # row's score accumulator) = 8.

            # ---- encoder loads (bf16, 1MB each, 1KB inner blocks) ----------
            def load_encbf(b):
                t = encbf_pool.tile([128, KH * S], bf16, tag="encbf",
                                    name=f"encbf{b}")
                nc.sync.dma_start(
                    t.rearrange("p (k s) -> p k s", s=S),
                    enc_t[b].rearrange("(k p) s -> p k s", p=128))
                return t

            et_bf = {b: load_encbf(b) for b in range(GB)}

            # ---- fp8 weight block for the main matmul (2MB, 2KB inner) ----
            w2t_sb = consts.tile([128, KH * F], fp8)
            nc.sync.dma_start(
                w2t_sb.rearrange("p (k f) -> p k f", f=F),
                w2t.rearrange("(k p) f -> p k f", p=128))

            # ---- small constants (shipped fp32, cast on device) ----
            ones8 = consts.tile([1, BL], bf16)
            nc.vector.memset(ones8[:], 1.0)
            b_attn_32 = consts.tile([1, F], f32)
            nc.sync.dma_start(b_attn_32[:], b_attn_d[:])
            b_attn_sb = consts.tile([1, F], bf16)
            nc.vector.tensor_copy(b_attn_sb[:], b_attn_32[:])
            b_comb_32 = consts.tile([1, H], f32)
            nc.sync.dma_start(b_comb_32[:], b_comb_d[:])
            b_comb_sb = consts.tile([1, H], bf16)
            nc.vector.tensor_copy(b_comb_sb[:], b_comb_32[:])

            hidT_32 = consts.tile([128, KH * BL], f32)
            decT_32 = consts.tile([128, KH * BL], f32)
            w2rep_32 = consts.tile([128, KF * BL], f32)
            for kc in range(KH):
                nc.sync.dma_start(hidT_32[:, kc * BL:(kc + 1) * BL],
                                  hidT[kc * 128:(kc + 1) * 128, :])
                nc.sync.dma_start(decT_32[:, kc * BL:(kc + 1) * BL],
                                  decT[kc * 128:(kc + 1) * 128, :])
            for ft in range(KF):
                nc.sync.dma_start(w2rep_32[:, ft * BL:(ft + 1) * BL],
                                  w2rep[ft * 128:(ft + 1) * 128, :])
            hidT_sb = consts.tile([128, KH * BL], bf16)
            nc.vector.tensor_copy(hidT_sb[:], hidT_32[:])
            decT_sb = consts.tile([128, KH * BL], bf16)
            nc.vector.tensor_copy(decT_sb[:], decT_32[:])
            w2rep_sb = consts.tile([128, KF * BL], bf16)
            nc.vector.tensor_copy(w2rep_sb[:], w2rep_32[:])

            # ---- preamble: hidbT[f, b] = (hidden @ W1h.T + b_attn)^T ------
            # operand-swapped: W1h chunks stationary, batch (8) moving;
            # output lands transposed (f on partitions) with no PE transpose.
            # ps_pre is one PSUM bank with 16 sub-bank slices. start=True
            # marks the whole 2KB zero-region pending-zero, so it may be
            # emitted exactly once (first touch); later slices first-touch
            # lazily zero their own bytes.
            hidbT_sb = consts.tile([128, KF * BL], f32)
            with tc.tile_pool(name="psPre", bufs=1, space="PSUM") as psPre_pool:
                ps_pre = psPre_pool.tile([128, KF * BL], f32, tag="pre",
                                         name="pspre")
                for kc in range(KH):
                    w1c = w1h_pool.tile([128, F], bf16, tag="w1h", name="w1c")
                    nc.sync.dma_start(w1c[:], w1h[kc * 128:(kc + 1) * 128, :])
                    for ft in range(KF):
                        nc.tensor.matmul(
                            ps_pre[:, ft * BL:(ft + 1) * BL],
                            w1c[:, ft * 128:(ft + 1) * 128],
                            hidT_sb[:, kc * BL:(kc + 1) * BL],
                            start=(kc == 0 and ft == 0), stop=False,
                            skip_group_check=True)
                for ft in range(KF):
                    nc.tensor.matmul(
                        ps_pre[:, ft * BL:(ft + 1) * BL],
                        b_attn_sb[:, ft * 128:(ft + 1) * 128],
                        ones8[:],
                        start=False, stop=(ft == KF - 1),
                        skip_group_check=True)
                nc.vector.tensor_copy(hidbT_sb[:], ps_pre[:])

            # ---- fp8 casts of the encoder tiles (GpSimd; idle engine) -----
            et_q = {}
            for b in range(GB):
                et_q[b] = encq_pool.tile([128, KH * S], fp8, tag="encq",
                                         name=f"encq{b}")
                nc.gpsimd.tensor_copy(et_q[b][:], et_bf[b][:])

            # ---- final combine, decoder_out half (no main-loop deps) ------
            # out^T[f, b] accumulates in one PSUM tile [128, 8fc*8b].
            psOut_pool = ctx.enter_context(
                tc.tile_pool(name="psOut", bufs=1, space="PSUM"))
            ps_out = psOut_pool.tile([128, 8 * BL], f32, tag="out",
                                     name="psout")
            # Same single-start rule as ps_pre: one bank, 8 sub-bank slices.
            for kc in range(KH):
                w = wct_pool.tile([128, H], bf16, tag="wct", name="wctt")
                nc.sync.dma_start(w[:], wct[kc * 128:(kc + 1) * 128, :])
                for fc in range(8):
                    nc.tensor.matmul(
                        ps_out[:, fc * BL:(fc + 1) * BL],
                        w[:, fc * 128:(fc + 1) * 128],
                        decT_sb[:, kc * BL:(kc + 1) * BL],
                        start=(kc == 0 and fc == 0), stop=False,
                        skip_group_check=True)

            # ---- remaining encoder loads + casts (group 2) ----------------
            for b in range(GB, BL):
                et_bf[b] = load_encbf(b)
            for b in range(GB, BL):
                et_q[b] = encq_pool.tile([128, KH * S], fp8, tag="encq",
                                         name=f"encq{b}")
                nc.gpsimd.tensor_copy(et_q[b][:], et_bf[b][:])

            # ---- shared tiles for softmax / scores ------------------------
            appT_sb = consts.tile([128, KH * BL], f32)
            appT_bf = consts.tile([128, KH * BL], bf16)

            def emit_scores_mm(psc, ft, b):
                # psc (all 8 rows identical) += w2(ft) . th(ft,b)
                nc.tensor.matmul(
                    psc[:],
                    w2rep_sb[:, ft * BL:(ft + 1) * BL],
                    th_tiles[(ft, b)][:],
                    start=(ft == 0), stop=(ft == KF - 1))
                del th_tiles[(ft, b)]

            def emit_softmax_apply(psc, b):
                negmax = small_pool.tile([BL, 1], f32, tag="negmax",
                                         name="negmax")
                nc.vector.reduce_max(negmax[:], psc[:], axis=AX.X,
                                     negate=True)
                attn = attn_pool.tile([BL, S], bf16, tag="attn", name="attn")
                sumexp = small_pool.tile([BL, 1], f32, tag="sumexp",
                                         name="sumexp")
                nc.scalar.activation(attn[:], psc[:], AF.Exp,
                                     bias=negmax[:], scale=1.0,
                                     accum_out=sumexp[:])
                recip = small_pool.tile([BL, 1], f32, tag="recip",
                                        name="recip")
                nc.vector.reciprocal(recip[:], sumexp[:])
                nc.vector.tensor_scalar_mul(attn[:], attn[:], recip[:])

                # broadcast attn row across 128 partitions via DRAM bounce
                attn_dr = dram_pool.tile([1, S], bf16, tag="attn_dr",
                                         name="attn_dr")
                nc.sync.dma_start(attn_dr[:], attn[0:1, :])
                abc = abc_pool.tile([128, S], bf16, tag="abc", name="abc")
                nc.sync.dma_start(abc[:],
                                  attn_dr[0:1, :].to_broadcast((128, S)))

                et = et_bf[b]
                for kc in range(KH):
                    scr = scr_pool.tile([128, S], bf16, tag="scr", name="scr")
                    nc.vector.tensor_tensor(
                        out=scr[:], in0=et[:, kc * S:(kc + 1) * S],
                        in1=abc[:], op=ALU.mult)
                    nc.vector.reduce_sum(
                        appT_sb[:, kc * BL + b: kc * BL + b + 1],
                        scr[:], axis=AX.X)
                nc.vector.tensor_copy(
                    appT_bf.rearrange("p (k b) -> p k b", b=BL)[:, :, b],
                    appT_sb.rearrange("p (k b) -> p k b", b=BL)[:, :, b])

            # ---- main sweep: NG groups of GB batch rows -------------------
            psT_pool = ctx.enter_context(
                tc.tile_pool(name="psT", bufs=4, space="PSUM"))
            psSc_pool = ctx.enter_context(
                tc.tile_pool(name="psSc", bufs=3, space="PSUM"))

            th_tiles = {}
            w2t_3d = w2t_sb.rearrange("p (k f) -> p k f", f=F)
            pending = []          # (ft, b) scores matmuls delayed one ft

            for g in range(NG):
                bs = range(g * GB, (g + 1) * GB)
                psc = {b: psSc_pool.tile([BL, S], f32, tag="psc",
                                         name=f"psc{b%GB}") for b in bs}
                psT = {}
                for ft in range(KF):
                    for kp in range(KH // 2):
                        # DoubleRow pair: contraction chunks 2kp, 2kp+1
                        lhs = w2t_3d[:, 2 * kp:2 * kp + 2,
                                     ft * 128:(ft + 1) * 128]
                        for b in bs:
                            if kp == 0:
                                psT[b] = psT_pool.tile(
                                    [128, S], f32, tag="pT", name=f"pT{b%GB}")
                            eq3 = et_q[b].rearrange("p (k s) -> p k s", s=S)
                            nc.tensor.matmul(
                                psT[b][:], lhs,
                                eq3[:, 2 * kp:2 * kp + 2, :],
                                start=(kp == 0), stop=(kp == KH // 2 - 1),
                                perf_mode=DR)
                    for b in bs:
                        t = th_pool.tile([128, S], bf16, tag="tanh",
                                         name="tanh")
                        nc.scalar.activation(
                            t[:], psT[b][:], AF.Tanh,
                            bias=hidbT_sb[:, ft * BL + b: ft * BL + b + 1],
                            scale=1.0 / WSCALE)
                        th_tiles[(ft, b)] = t
                    # scores run one ft behind so tanh is off the PE path
                    for (pft, pb) in pending:
                        emit_scores_mm(psc[pb], pft, pb)
                    pending = [(ft, b) for b in bs]
                # last ft's scores, then softmax/apply for this group
                # (the V/S/DMA chain overlaps the next group's PE sweep)
                for (pft, pb) in pending:
                    emit_scores_mm(psc[pb], pft, pb)
                pending = []
                for b in bs:
                    emit_softmax_apply(psc[b], b)

            # ---- final combine, applied half + bias + tanh ----------------
            for kc in range(KH):
                w = wct_pool.tile([128, H], bf16, tag="wct", name="wctt")
                nc.sync.dma_start(w[:], wct[(KH + kc) * 128:
                                            (KH + kc + 1) * 128, :])
                for fc in range(8):
                    nc.tensor.matmul(
                        ps_out[:, fc * BL:(fc + 1) * BL],
                        w[:, fc * 128:(fc + 1) * 128],
                        appT_bf[:, kc * BL:(kc + 1) * BL],
                        start=False, stop=False,
                        skip_group_check=True)
            for fc in range(8):
                nc.tensor.matmul(
                    ps_out[:, fc * BL:(fc + 1) * BL],
                    b_comb_sb[:, fc * 128:(fc + 1) * 128],
                    ones8[:],
                    start=False, stop=(fc == 7),
                    skip_group_check=True)

            outT_sb = consts.tile([128, 8 * BL], f32)
            nc.scalar.activation(outT_sb[:], ps_out[:], AF.Tanh)
            for fc in range(8):
                nc.sync.dma_start(outT_d[fc * 128:(fc + 1) * 128, :],
                                  outT_sb[:, fc * BL:(fc + 1) * BL])
            for kc in range(KH):
                nc.sync.dma_start(appT_d[kc * 128:(kc + 1) * 128, :],
                                  appT_sb[:, kc * BL:(kc + 1) * BL])

    nc.compile()
    return nc


def _get_nc():
    if "nc" not in _CACHE:
        _CACHE["nc"] = _build()
    return _CACHE["nc"]


def make_in_maps(inputs):
    import ml_dtypes
    bf = ml_dtypes.bfloat16
    f8 = ml_dtypes.float8_e4m3fn

    inp = {k: np.asarray(v, dtype=np.float32) for k, v in inputs.items()}
    hidden = inp["hidden"]
    decoder_out = inp["decoder_out"]
    encoder_states = inp["encoder_states"]
    W_attn = inp["W_attn"]
    b_attn = inp["b_attn"]
    W_attn2 = inp["W_attn2"]
    W_comb = inp["W_comb"]
    b_comb = inp["b_comb"]
    # b_attn2 shifts every score equally -> softmax-invariant, unused.

    watT = np.ascontiguousarray(W_attn.T)              # [2H, 2H]
    w1h = watT[:H].astype(bf)                          # hidden-half, bf16
    w2t = np.ascontiguousarray(watT[H:] * WSCALE).astype(f8)  # enc-half, fp8
    wct = np.ascontiguousarray(W_comb.T).astype(bf)
    w2rep = np.ascontiguousarray(np.repeat(W_attn2.reshape(F, 1), BL, axis=1))
    b_attn_2d = np.ascontiguousarray(b_attn.reshape(1, F))
    b_comb_2d = np.ascontiguousarray(b_comb.reshape(1, H))

    in_maps = []
    for c in range(NCORES):
        sl = slice(c * BL, (c + 1) * BL)
        in_maps.append({
            "enc_t": np.ascontiguousarray(
                encoder_states[:, sl, :].transpose(1, 2, 0)).astype(bf),
            "w1h": w1h,
            "w2t": w2t,
            "wct": wct,
            "hidT": np.ascontiguousarray(hidden[sl].T),
            "decT": np.ascontiguousarray(decoder_out[sl].T),
            "w2rep": w2rep,
            "b_attn": b_attn_2d,
            "b_comb": b_comb_2d,
        })
    return in_maps


def kernel(**inputs):
    from concourse.bass_utils import run_bass_kernel_spmd

    in_maps = make_in_maps(inputs)
    nc = _get_nc()
    res = run_bass_kernel_spmd(nc, in_maps, list(range(NCORES)))
    out = np.concatenate(
        [np.ascontiguousarray(res.results[c]["outT"].T)
         for c in range(NCORES)], axis=0)
    applied = np.concatenate(
        [np.ascontiguousarray(res.results[c]["appliedT"].T)
         for c in range(NCORES)], axis=0)
    return out.astype(np.float32), applied.astype(np.float32)


# revision 20
# speedup vs baseline: 1.3723x; 1.0320x over previous
"""Trainium2 Bass kernel for nn_AttentionModule (Bahdanau-style attention).

Reference computation (S=512, B=64, H=1024, F=2H):
    cat    = concat([hidden bcast to (S,B,H), encoder_states], -1)      [S,B,2H]
    scores = tanh(cat @ W_attn.T + b_attn) @ W_attn2.T + b_attn2        [S,B,1]
    attn   = softmax(scores[..., 0].T, axis=-1)                         [B,S]
    applied= einsum("bs,sbh->bh", attn, encoder_states)                 [B,H]
    out    = tanh(concat([decoder_out, applied], -1) @ W_comb.T + b_comb)

Sharding: data-parallel over B across 8 cores (8 batch rows per core).

Optimized structure (vs the bf16 baseline):
  - The dominant matmul T[f,s] = W1e @ enc (per b: [2048,1024]@[1024,512])
    runs in fp8e4 with perf_mode=DoubleRow: each instruction consumes two
    128-deep contraction chunks at once (~1.8x the bf16 rate).  W1e ships
    from the host pre-scaled by 256 (fp8e4 subnormal range) and the tanh
    activation un-scales with scale=1/256.  enc ships bf16 (needed at
    bf16 precision for the attention application) and is cast to fp8 on
    GpSimd, which is otherwise idle.
  - Batch rows are processed in two groups of 4 with b innermost so each
    DoubleRow weight load is reused by 4 matmuls (LDWEIGHTS has no FWL in
    DoubleRow mode, so un-reused loads cost ~256 PE cycles each).
  - The scores matmul (contract f: w2 . tanh) accumulates inline, one
    partial matmul per (ft,b) emitted one ft behind the main sweep, into
    per-b PSUM slices packed 4-to-a-bank at partition offsets 0/32/64/96.
  - hid @ W1h.T preamble and the final combine run operand-swapped
    (weights stationary, batch as the 8-wide moving operand) producing
    transposed outputs directly -- no PE transposes, and the decoder_out
    half of the final combine is emitted early, off the critical tail.
  - softmax / attention-apply (VectorE mult+reduce over resident bf16
    enc tiles) unchanged from baseline, overlapping the next group's PE
    sweep; attention row broadcast via DRAM bounce.

Known pitfalls baked in (from the baseline, still honored):
  - small tensors ship fp32 and are cast on device (tiny bf16 rows get
    corrupted host->device).
  - multi-dim rearrange DMAs only with >=1KB inner blocks (encoder/w2t).
  - 16/32-bit matmul operand mixing rejected; fp8 pairs must both be fp8.
"""

import numpy as np

S, B, H = 512, 64, 1024
F = 2 * H
NCORES = 8
BL = B // NCORES          # 8 batch rows per core
KH = H // 128             # 8 contraction chunks over H
KF = F // 128             # 16 feature tiles
GB = 2                    # batch rows per group (PSUM-bank limited: per-b
                          # score slices pack at partition offsets 0/32 only;
                          # quadrant 3 (offset 96) is unusable on TRN2)
NG = BL // GB             # 2 groups
WSCALE = 256.0            # fp8 weight pre-scale (power of 2)

_CACHE = {}


def _build(num_devices=NCORES):
    from contextlib import ExitStack

    import concourse.tile as tile
    from concourse import bacc, mybir

    f32 = mybir.dt.float32
    bf16 = mybir.dt.bfloat16
    fp8 = mybir.dt.float8e4
    AF = mybir.ActivationFunctionType
    ALU = mybir.AluOpType
    AX = mybir.AxisListType
    DR = mybir.MatmulPerfMode.DoubleRow

    nc = bacc.Bacc("TRN2", target_bir_lowering=False, debug=False,
                   num_devices=num_devices)

    enc_t = nc.dram_tensor("enc_t", [BL, H, S], bf16, kind="ExternalInput").ap()
    # fp8 encoder copy, host-pre-arranged to [b, partition, kc*S+s] so the
    # load is a plain 2D DMA with 4KB-contiguous rows per partition.
    enc_q_d = nc.dram_tensor("enc_q", [BL, 128, KH * S], fp8,
                             kind="ExternalInput").ap()
    w1h = nc.dram_tensor("w1h", [H, F], bf16, kind="ExternalInput").ap()
    w2t = nc.dram_tensor("w2t", [H, F], fp8, kind="ExternalInput").ap()
    wct = nc.dram_tensor("wct", [F, H], bf16, kind="ExternalInput").ap()
    hidT = nc.dram_tensor("hidT", [H, BL], f32, kind="ExternalInput").ap()
    decT = nc.dram_tensor("decT", [H, BL], f32, kind="ExternalInput").ap()
    w2rep = nc.dram_tensor("w2rep", [F, BL], f32, kind="ExternalInput").ap()
    b_attn_d = nc.dram_tensor("b_attn", [1, F], f32, kind="ExternalInput").ap()
    b_comb_d = nc.dram_tensor("b_comb", [1, H], f32, kind="ExternalInput").ap()
    outT_d = nc.dram_tensor("outT", [H, BL], f32, kind="ExternalOutput").ap()
    appT_d = nc.dram_tensor("appliedT", [H, BL], f32,
                            kind="ExternalOutput").ap()

    with tile.TileContext(nc) as tc:
        with ExitStack() as ctx:
            consts = ctx.enter_context(tc.tile_pool(name="consts", bufs=1))
            encbf_pool = ctx.enter_context(tc.tile_pool(name="encbf", bufs=BL))
            encq_pool = ctx.enter_context(tc.tile_pool(name="encq", bufs=BL))
            w1h_pool = ctx.enter_context(tc.tile_pool(name="w1h", bufs=2))
            wct_pool = ctx.enter_context(tc.tile_pool(name="wct", bufs=4))
            th_pool = ctx.enter_context(tc.tile_pool(name="th", bufs=10))
            attn_pool = ctx.enter_context(tc.tile_pool(name="attn", bufs=2))
            abc_pool = ctx.enter_context(tc.tile_pool(name="abc", bufs=2))
            scr_pool = ctx.enter_context(tc.tile_pool(name="scr", bufs=2))
            small_pool = ctx.enter_context(tc.tile_pool(name="small", bufs=8))
            dram_pool = ctx.enter_context(
                tc.tile_pool(name="dram", bufs=2, space="DRAM"))
            # PSUM budget (8 banks): psPre(1) closes before the main sweep,
            # then psOut(1) + psT(4) + psSc(3, one [8,512] bank per batch
            # row's score accumulator) = 8.

            # ---- encoder loads -------------------------------------------
            def load_encbf(b):
                t = encbf_pool.tile([128, KH * S], bf16, tag="encbf",
                                    name=f"encbf{b}")
                nc.sync.dma_start(
                    t.rearrange("p (k s) -> p k s", s=S),
                    enc_t[b].rearrange("(k p) s -> p k s", p=128))
                return t

            def load_encq(b):
                # 4 chunk DMAs (one per kc-pair) so the first matmul of a
                # group is gated on 256KB, not the full tile.
                t = encq_pool.tile([128, KH * S], fp8, tag="encq",
                                   name=f"encq{b}")
                for kp in range(KH // 2):
                    c0, c1 = 2 * kp * S, (2 * kp + 2) * S
                    nc.sync.dma_start(t[:, c0:c1], enc_q_d[b][:, c0:c1])
                return t

            # fp8 weight block, split per kc chunk for fine-grained deps
            w2t_sb = consts.tile([128, KH * F], fp8)
            for kc in range(KH):
                nc.sync.dma_start(w2t_sb[:, kc * F:(kc + 1) * F],
                                  w2t[kc * 128:(kc + 1) * 128, :])
            et_q = {b: load_encq(b) for b in range(2 * GB)}
            et_bf = {b: load_encbf(b) for b in range(GB)}

            # ---- small constants (shipped fp32, cast on device) ----
            ones8 = consts.tile([1, BL], bf16)
            nc.vector.memset(ones8[:], 1.0)
            b_attn_32 = consts.tile([1, F], f32)
            nc.sync.dma_start(b_attn_32[:], b_attn_d[:])
            b_attn_sb = consts.tile([1, F], bf16)
            nc.vector.tensor_copy(b_attn_sb[:], b_attn_32[:])
            b_comb_32 = consts.tile([1, H], f32)
            nc.sync.dma_start(b_comb_32[:], b_comb_d[:])
            b_comb_sb = consts.tile([1, H], bf16)
            nc.vector.tensor_copy(b_comb_sb[:], b_comb_32[:])

            hidT_32 = consts.tile([128, KH * BL], f32)
            decT_32 = consts.tile([128, KH * BL], f32)
            w2rep_32 = consts.tile([128, KF * BL], f32)
            for kc in range(KH):
                nc.sync.dma_start(hidT_32[:, kc * BL:(kc + 1) * BL],
                                  hidT[kc * 128:(kc + 1) * 128, :])
                nc.sync.dma_start(decT_32[:, kc * BL:(kc + 1) * BL],
                                  decT[kc * 128:(kc + 1) * 128, :])
            for ft in range(KF):
                nc.sync.dma_start(w2rep_32[:, ft * BL:(ft + 1) * BL],
                                  w2rep[ft * 128:(ft + 1) * 128, :])
            hidT_sb = consts.tile([128, KH * BL], bf16)
            nc.vector.tensor_copy(hidT_sb[:], hidT_32[:])
            decT_sb = consts.tile([128, KH * BL], bf16)
            nc.vector.tensor_copy(decT_sb[:], decT_32[:])
            w2rep_sb = consts.tile([128, KF * BL], bf16)
            nc.vector.tensor_copy(w2rep_sb[:], w2rep_32[:])

            # ---- preamble: hidbT[f, b] = (hidden @ W1h.T + b_attn)^T ------
            # operand-swapped: W1h chunks stationary, batch (8) moving;
            # output lands transposed (f on partitions) with no PE transpose.
            # ps_pre is one PSUM bank with 16 sub-bank slices. start=True
            # marks the whole 2KB zero-region pending-zero, so it may be
            # emitted exactly once (first touch); later slices first-touch
            # lazily zero their own bytes.
            hidbT_sb = consts.tile([128, KF * BL], f32)
            with tc.tile_pool(name="psPre", bufs=1, space="PSUM") as psPre_pool:
                ps_pre = psPre_pool.tile([128, KF * BL], f32, tag="pre",
                                         name="pspre")
                for kc in range(KH):
                    w1c = w1h_pool.tile([128, F], bf16, tag="w1h", name="w1c")
                    nc.sync.dma_start(w1c[:], w1h[kc * 128:(kc + 1) * 128, :])
                    for ft in range(KF):
                        nc.tensor.matmul(
                            ps_pre[:, ft * BL:(ft + 1) * BL],
                            w1c[:, ft * 128:(ft + 1) * 128],
                            hidT_sb[:, kc * BL:(kc + 1) * BL],
                            start=(kc == 0 and ft == 0), stop=False,
                            skip_group_check=True)
                for ft in range(KF):
                    nc.tensor.matmul(
                        ps_pre[:, ft * BL:(ft + 1) * BL],
                        b_attn_sb[:, ft * 128:(ft + 1) * 128],
                        ones8[:],
                        start=False, stop=(ft == KF - 1),
                        skip_group_check=True)
                nc.vector.tensor_copy(hidbT_sb[:], ps_pre[:])

            # ---- final combine, decoder_out half (no main-loop deps) ------
            # out^T[f, b] accumulates in one PSUM tile [128, 8fc*8b].
            psOut_pool = ctx.enter_context(
                tc.tile_pool(name="psOut", bufs=1, space="PSUM"))
            ps_out = psOut_pool.tile([128, 8 * BL], f32, tag="out",
                                     name="psout")
            # Same single-start rule as ps_pre: one bank, 8 sub-bank slices.
            for kc in range(KH):
                w = wct_pool.tile([128, H], bf16, tag="wct", name="wctt")
                nc.sync.dma_start(w[:], wct[kc * 128:(kc + 1) * 128, :])
                for fc in range(8):
                    nc.tensor.matmul(
                        ps_out[:, fc * BL:(fc + 1) * BL],
                        w[:, fc * 128:(fc + 1) * 128],
                        decT_sb[:, kc * BL:(kc + 1) * BL],
                        start=(kc == 0 and fc == 0), stop=False,
                        skip_group_check=True)

            # ---- remaining encoder loads ----------------------------------
            for b in range(2 * GB, BL):
                et_q[b] = load_encq(b)
            for b in range(GB, BL):
                et_bf[b] = load_encbf(b)

            # ---- shared tiles for softmax / scores ------------------------
            appT_sb = consts.tile([128, KH * BL], f32)
            appT_bf = consts.tile([128, KH * BL], bf16)

            def emit_scores_mm(psc, ft, b):
                # psc (all 8 rows identical) += w2(ft) . th(ft,b)
                nc.tensor.matmul(
                    psc[:],
                    w2rep_sb[:, ft * BL:(ft + 1) * BL],
                    th_tiles[(ft, b)][:],
                    start=(ft == 0), stop=(ft == KF - 1))
                del th_tiles[(ft, b)]

            def emit_softmax_apply(psc, b):
                negmax = small_pool.tile([BL, 1], f32, tag="negmax",
                                         name="negmax")
                nc.vector.reduce_max(negmax[:], psc[:], axis=AX.X,
                                     negate=True)
                attn = attn_pool.tile([BL, S], bf16, tag="attn", name="attn")
                sumexp = small_pool.tile([BL, 1], f32, tag="sumexp",
                                         name="sumexp")
                nc.scalar.activation(attn[:], psc[:], AF.Exp,
                                     bias=negmax[:], scale=1.0,
                                     accum_out=sumexp[:])
                recip = small_pool.tile([BL, 1], f32, tag="recip",
                                        name="recip")
                nc.vector.reciprocal(recip[:], sumexp[:])
                nc.vector.tensor_scalar_mul(attn[:], attn[:], recip[:])

                # broadcast attn row across 128 partitions via DRAM bounce
                attn_dr = dram_pool.tile([1, S], bf16, tag="attn_dr",
                                         name="attn_dr")
                nc.sync.dma_start(attn_dr[:], attn[0:1, :])
                abc = abc_pool.tile([128, S], bf16, tag="abc", name="abc")
                nc.sync.dma_start(abc[:],
                                  attn_dr[0:1, :].to_broadcast((128, S)))

                et = et_bf[b]
                for kc in range(KH):
                    scr = scr_pool.tile([128, S], bf16, tag="scr", name="scr")
                    nc.vector.tensor_tensor(
                        out=scr[:], in0=et[:, kc * S:(kc + 1) * S],
                        in1=abc[:], op=ALU.mult)
                    nc.vector.reduce_sum(
                        appT_sb[:, kc * BL + b: kc * BL + b + 1],
                        scr[:], axis=AX.X)
                nc.vector.tensor_copy(
                    appT_bf.rearrange("p (k b) -> p k b", b=BL)[:, :, b],
                    appT_sb.rearrange("p (k b) -> p k b", b=BL)[:, :, b])

            # ---- main sweep: NG groups of GB batch rows -------------------
            psT_pool = ctx.enter_context(
                tc.tile_pool(name="psT", bufs=4, space="PSUM"))
            psSc_pool = ctx.enter_context(
                tc.tile_pool(name="psSc", bufs=3, space="PSUM"))

            th_tiles = {}
            w2t_3d = w2t_sb.rearrange("p (k f) -> p k f", f=F)
            pending = []          # (ft, b) scores matmuls delayed one ft

            for g in range(NG):
                bs = range(g * GB, (g + 1) * GB)
                psc = {b: psSc_pool.tile([BL, S], f32, tag="psc",
                                         name=f"psc{b%GB}") for b in bs}
                psT = {}
                for ft in range(KF):
                    for kp in range(KH // 2):
                        # DoubleRow pair: contraction chunks 2kp, 2kp+1
                        lhs = w2t_3d[:, 2 * kp:2 * kp + 2,
                                     ft * 128:(ft + 1) * 128]
                        for b in bs:
                            if kp == 0:
                                psT[b] = psT_pool.tile(
                                    [128, S], f32, tag="pT", name=f"pT{b%GB}")
                            eq3 = et_q[b].rearrange("p (k s) -> p k s", s=S)
                            nc.tensor.matmul(
                                psT[b][:], lhs,
                                eq3[:, 2 * kp:2 * kp + 2, :],
                                start=(kp == 0), stop=(kp == KH // 2 - 1),
                                perf_mode=DR)
                    for b in bs:
                        t = th_pool.tile([128, S], bf16, tag="tanh",
                                         name="tanh")
                        nc.scalar.activation(
                            t[:], psT[b][:], AF.Tanh,
                            bias=hidbT_sb[:, ft * BL + b: ft * BL + b + 1],
                            scale=1.0 / WSCALE)
                        th_tiles[(ft, b)] = t
                    # scores run one ft behind so tanh is off the PE path
                    for (pft, pb) in pending:
                        emit_scores_mm(psc[pb], pft, pb)
                    pending = [(ft, b) for b in bs]
                # last ft's scores, then softmax/apply for this group
                # (the V/S/DMA chain overlaps the next group's PE sweep)
                for (pft, pb) in pending:
                    emit_scores_mm(psc[pb], pft, pb)
                pending = []
                for b in bs:
                    emit_softmax_apply(psc[b], b)

            # ---- final combine, applied half + bias + tanh ----------------
            for kc in range(KH):
                w = wct_pool.tile([128, H], bf16, tag="wct", name="wctt")
                nc.sync.dma_start(w[:], wct[(KH + kc) * 128:
                                            (KH + kc + 1) * 128, :])
                for fc in range(8):
                    nc.tensor.matmul(
                        ps_out[:, fc * BL:(fc + 1) * BL],
                        w[:, fc * 128:(fc + 1) * 128],
                        appT_bf[:, kc * BL:(kc + 1) * BL],
                        start=False, stop=False,
                        skip_group_check=True)
            for fc in range(8):
                nc.tensor.matmul(
                    ps_out[:, fc * BL:(fc + 1) * BL],
                    b_comb_sb[:, fc * 128:(fc + 1) * 128],
                    ones8[:],
                    start=False, stop=(fc == 7),
                    skip_group_check=True)

            outT_sb = consts.tile([128, 8 * BL], f32)
            nc.scalar.activation(outT_sb[:], ps_out[:], AF.Tanh)
            for fc in range(8):
                nc.sync.dma_start(outT_d[fc * 128:(fc + 1) * 128, :],
                                  outT_sb[:, fc * BL:(fc + 1) * BL])
            for kc in range(KH):
                nc.sync.dma_start(appT_d[kc * 128:(kc + 1) * 128, :],
                                  appT_sb[:, kc * BL:(kc + 1) * BL])

    nc.compile()
    return nc


def _get_nc():
    if "nc" not in _CACHE:
        _CACHE["nc"] = _build()
    return _CACHE["nc"]


def make_in_maps(inputs):
    import ml_dtypes
    bf = ml_dtypes.bfloat16
    f8 = ml_dtypes.float8_e4m3fn

    inp = {k: np.asarray(v, dtype=np.float32) for k, v in inputs.items()}
    hidden = inp["hidden"]
    decoder_out = inp["decoder_out"]
    encoder_states = inp["encoder_states"]
    W_attn = inp["W_attn"]
    b_attn = inp["b_attn"]
    W_attn2 = inp["W_attn2"]
    W_comb = inp["W_comb"]
    b_comb = inp["b_comb"]
    # b_attn2 shifts every score equally -> softmax-invariant, unused.

    watT = np.ascontiguousarray(W_attn.T)              # [2H, 2H]
    w1h = watT[:H].astype(bf)                          # hidden-half, bf16
    w2t = np.ascontiguousarray(watT[H:] * WSCALE).astype(f8)  # enc-half, fp8
    wct = np.ascontiguousarray(W_comb.T).astype(bf)
    w2rep = np.ascontiguousarray(np.repeat(W_attn2.reshape(F, 1), BL, axis=1))
    b_attn_2d = np.ascontiguousarray(b_attn.reshape(1, F))
    b_comb_2d = np.ascontiguousarray(b_comb.reshape(1, H))

    in_maps = []
    for c in range(NCORES):
        sl = slice(c * BL, (c + 1) * BL)
        enc_c = np.ascontiguousarray(
            encoder_states[:, sl, :].transpose(1, 2, 0))      # [BL, H, S]
        enc_q = np.ascontiguousarray(
            enc_c.reshape(BL, KH, 128, S).transpose(0, 2, 1, 3)
            .reshape(BL, 128, KH * S)).astype(f8)             # [BL, p, k*s]
        in_maps.append({
            "enc_t": enc_c.astype(bf),
            "enc_q": enc_q,
            "w1h": w1h,
            "w2t": w2t,
            "wct": wct,
            "hidT": np.ascontiguousarray(hidden[sl].T),
            "decT": np.ascontiguousarray(decoder_out[sl].T),
            "w2rep": w2rep,
            "b_attn": b_attn_2d,
            "b_comb": b_comb_2d,
        })
    return in_maps


def kernel(**inputs):
    from concourse.bass_utils import run_bass_kernel_spmd

    in_maps = make_in_maps(inputs)
    nc = _get_nc()
    res = run_bass_kernel_spmd(nc, in_maps, list(range(NCORES)))
    out = np.concatenate(
        [np.ascontiguousarray(res.results[c]["outT"].T)
         for c in range(NCORES)], axis=0)
    applied = np.concatenate(
        [np.ascontiguousarray(res.results[c]["appliedT"].T)
         for c in range(NCORES)], axis=0)
    return out.astype(np.float32), applied.astype(np.float32)


# revision 27
# speedup vs baseline: 1.3848x; 1.0091x over previous
"""Trainium2 Bass kernel for nn_AttentionModule (Bahdanau-style attention).

Reference computation (S=512, B=64, H=1024, F=2H):
    cat    = concat([hidden bcast to (S,B,H), encoder_states], -1)      [S,B,2H]
    scores = tanh(cat @ W_attn.T + b_attn) @ W_attn2.T + b_attn2        [S,B,1]
    attn   = softmax(scores[..., 0].T, axis=-1)                         [B,S]
    applied= einsum("bs,sbh->bh", attn, encoder_states)                 [B,H]
    out    = tanh(concat([decoder_out, applied], -1) @ W_comb.T + b_comb)

Sharding: data-parallel over B across 8 cores (8 batch rows per core).

Optimized structure (vs the bf16 baseline):
  - The dominant matmul T[f,s] = W1e @ enc (per b: [2048,1024]@[1024,512])
    runs in fp8e4 with perf_mode=DoubleRow: each instruction consumes two
    128-deep contraction chunks at once (~1.8x the bf16 rate).  W1e and
    W1h ship pre-scaled by 256 (fp8e4 subnormal range); the consumers
    un-scale with activation scale=1/256.  The encoder ships twice: bf16
    [b,(kc p),s] for the attention apply, and fp8 pre-arranged to
    [b,p,(kc s)] (plain 2D DMA, 4KB rows) for the matmul.
  - Batch rows are processed in groups of 2 with b innermost so each
    DoubleRow weight load is reused (LDWEIGHTS has no FWL in DoubleRow
    mode).  PSUM budget (8 banks): psPre(1) closes after the preamble,
    then psT(4) + psSc(3: per-b score accumulator banks) + psOut(1).
  - The scores matmul (contract f: w2 . tanh) accumulates inline, one
    partial matmul per (ft,b) emitted one ft behind the main sweep.
  - hid @ W1h.T preamble and the final combine run operand-swapped
    (weights stationary, batch as the 8-wide moving operand) producing
    transposed outputs directly -- no PE transposes.  The decoder_out
    half of the final combine is emitted mid-sweep, off the critical
    tail.
  - DMA emission order is critical-first: small constants, w1h, w2t,
    first groups' fp8 encoder; the bf16 encoder tiles and wct stream in
    during the sweep.
  - softmax -> DRAM-bounce broadcast -> apply; the apply is a fused
    multiply+reduce (tensor_tensor_reduce) on VectorE for kc 1..7 and
    plain mult+reduce on GpSimd for kc 0, so the exposed tail after the
    last group is short.

Known pitfalls baked in (from the baseline, still honored):
  - small tensors ship fp32 and are cast on device (tiny bf16 rows get
    corrupted host->device).
  - multi-dim rearrange DMAs only with >=1KB inner blocks.
  - 16/32-bit matmul operand mixing rejected; fp8 pairs must both be fp8.
  - PSUM zero regions are 2KB: at most one accumulation group per bank
    region, and sub-bank slices may emit start=True only on first touch.
"""

import numpy as np

S, B, H = 512, 64, 1024
F = 2 * H
NCORES = 8
BL = B // NCORES          # 8 batch rows per core
KH = H // 128             # 8 contraction chunks over H
KF = F // 128             # 16 feature tiles
GB = 2                    # batch rows per group (PSUM-bank limited)
NG = BL // GB             # 4 groups
WSCALE = 256.0            # fp8 weight pre-scale (power of 2)

_CACHE = {}


def _build(num_devices=NCORES):
    from contextlib import ExitStack

    import concourse.tile as tile
    from concourse import bacc, mybir

    f32 = mybir.dt.float32
    bf16 = mybir.dt.bfloat16
    fp8 = mybir.dt.float8e4
    AF = mybir.ActivationFunctionType
    ALU = mybir.AluOpType
    AX = mybir.AxisListType
    DR = mybir.MatmulPerfMode.DoubleRow

    nc = bacc.Bacc("TRN2", target_bir_lowering=False, debug=False,
                   num_devices=num_devices)

    enc_t = nc.dram_tensor("enc_t", [BL, H, S], bf16, kind="ExternalInput").ap()
    # fp8 encoder copy, host-pre-arranged to [b, partition, kc*S+s] so the
    # load is a plain 2D DMA with 4KB-contiguous rows per partition.
    enc_q_d = nc.dram_tensor("enc_q", [BL, 128, KH * S], fp8,
                             kind="ExternalInput").ap()
    w1h = nc.dram_tensor("w1h", [H, F], fp8, kind="ExternalInput").ap()
    w2t = nc.dram_tensor("w2t", [H, F], fp8, kind="ExternalInput").ap()
    wct = nc.dram_tensor("wct", [F, H], bf16, kind="ExternalInput").ap()
    hidT = nc.dram_tensor("hidT", [H, BL], f32, kind="ExternalInput").ap()
    decT = nc.dram_tensor("decT", [H, BL], f32, kind="ExternalInput").ap()
    w2rep = nc.dram_tensor("w2rep", [F, BL], f32, kind="ExternalInput").ap()
    # b_attn ships pre-multiplied by WSCALE (folded into the x256-scaled
    # preamble PSUM, un-scaled together with it).
    b_attn_d = nc.dram_tensor("b_attn", [1, F], f32, kind="ExternalInput").ap()
    b_comb_d = nc.dram_tensor("b_comb", [1, H], f32, kind="ExternalInput").ap()
    outT_d = nc.dram_tensor("outT", [H, BL], f32, kind="ExternalOutput").ap()
    appT_d = nc.dram_tensor("appliedT", [H, BL], f32,
                            kind="ExternalOutput").ap()

    with tile.TileContext(nc) as tc:
        with ExitStack() as ctx:
            consts = ctx.enter_context(tc.tile_pool(name="consts", bufs=1))
            encbf_pool = ctx.enter_context(tc.tile_pool(name="encbf", bufs=BL))
            encq_pool = ctx.enter_context(tc.tile_pool(name="encq", bufs=BL))
            w1h_pool = ctx.enter_context(tc.tile_pool(name="w1h", bufs=2))
            wct_pool = ctx.enter_context(tc.tile_pool(name="wct", bufs=4))
            th_pool = ctx.enter_context(tc.tile_pool(name="th", bufs=10))
            attn_pool = ctx.enter_context(tc.tile_pool(name="attn", bufs=2))
            abc_pool = ctx.enter_context(tc.tile_pool(name="abc", bufs=2))
            scr_pool = ctx.enter_context(tc.tile_pool(name="scr", bufs=2))
            small_pool = ctx.enter_context(tc.tile_pool(name="small", bufs=8))
            dram_pool = ctx.enter_context(
                tc.tile_pool(name="dram", bufs=2, space="DRAM"))

            # ---- tiny constants first (feed the preamble) -----------------
            ones8 = consts.tile([1, BL], bf16)
            nc.vector.memset(ones8[:], 1.0)
            b_attn_32 = consts.tile([1, F], f32)
            nc.sync.dma_start(b_attn_32[:], b_attn_d[:])
            b_attn_sb = consts.tile([1, F], bf16)
            nc.vector.tensor_copy(b_attn_sb[:], b_attn_32[:])
            b_comb_32 = consts.tile([1, H], f32)
            nc.sync.dma_start(b_comb_32[:], b_comb_d[:])
            b_comb_sb = consts.tile([1, H], bf16)
            nc.vector.tensor_copy(b_comb_sb[:], b_comb_32[:])

            hidT_32 = consts.tile([128, KH * BL], f32)
            decT_32 = consts.tile([128, KH * BL], f32)
            w2rep_32 = consts.tile([128, KF * BL], f32)
            for kc in range(KH):
                nc.sync.dma_start(hidT_32[:, kc * BL:(kc + 1) * BL],
                                  hidT[kc * 128:(kc + 1) * 128, :])
            for kc in range(KH):
                nc.sync.dma_start(decT_32[:, kc * BL:(kc + 1) * BL],
                                  decT[kc * 128:(kc + 1) * 128, :])
            for ft in range(KF):
                nc.sync.dma_start(w2rep_32[:, ft * BL:(ft + 1) * BL],
                                  w2rep[ft * 128:(ft + 1) * 128, :])
            hidT_q = consts.tile([128, KH * BL], fp8)
            nc.vector.tensor_copy(hidT_q[:], hidT_32[:])
            decT_sb = consts.tile([128, KH * BL], bf16)
            nc.vector.tensor_copy(decT_sb[:], decT_32[:])
            w2rep_sb = consts.tile([128, KF * BL], bf16)
            nc.vector.tensor_copy(w2rep_sb[:], w2rep_32[:])

            # ---- critical-path DMAs: w2t chunks, first groups' fp8 enc ----
            def load_encq(b):
                # 4 chunk DMAs (one per kc-pair) for fine-grained deps
                t = encq_pool.tile([128, KH * S], fp8, tag="encq",
                                   name=f"encq{b}")
                for kp in range(KH // 2):
                    c0, c1 = 2 * kp * S, (2 * kp + 2) * S
                    nc.sync.dma_start(t[:, c0:c1], enc_q_d[b][:, c0:c1])
                return t

            def load_encbf(b):
                t = encbf_pool.tile([128, KH * S], bf16, tag="encbf",
                                    name=f"encbf{b}")
                nc.sync.dma_start(
                    t.rearrange("p (k s) -> p k s", s=S),
                    enc_t[b].rearrange("(k p) s -> p k s", p=128))
                return t

            w2t_sb = consts.tile([128, KH * F], fp8)
            for kc in range(KH):
                nc.sync.dma_start(w2t_sb[:, kc * F:(kc + 1) * F],
                                  w2t[kc * 128:(kc + 1) * 128, :])
            et_q = {b: load_encq(b) for b in range(2 * GB)}
            et_bf = {}

            # ---- preamble: hidbT[f, b] = (hidden @ W1h.T + b_attn)^T ------
            # operand-swapped: W1h chunks stationary (fp8, x256), batch (8)
            # moving; output lands transposed (f on partitions), un-scaled
            # by the copy-out.  ps_pre is one PSUM bank with 16 sub-bank
            # slices: start=True may be emitted only on first touch (2KB
            # zero region); later slices lazily zero their own bytes.
            hidbT_sb = consts.tile([128, KF * BL], f32)
            with tc.tile_pool(name="psPre", bufs=1, space="PSUM") as psPre_pool:
                ps_pre = psPre_pool.tile([128, KF * BL], f32, tag="pre",
                                         name="pspre")
                for kc in range(KH):
                    w1c = w1h_pool.tile([128, F], fp8, tag="w1h", name="w1c")
                    nc.sync.dma_start(w1c[:], w1h[kc * 128:(kc + 1) * 128, :])
                    for ft in range(KF):
                        nc.tensor.matmul(
                            ps_pre[:, ft * BL:(ft + 1) * BL],
                            w1c[:, ft * 128:(ft + 1) * 128],
                            hidT_q[:, kc * BL:(kc + 1) * BL],
                            start=(kc == 0 and ft == 0), stop=False,
                            skip_group_check=True)
                for ft in range(KF):
                    nc.tensor.matmul(
                        ps_pre[:, ft * BL:(ft + 1) * BL],
                        b_attn_sb[:, ft * 128:(ft + 1) * 128],
                        ones8[:],
                        start=False, stop=(ft == KF - 1),
                        skip_group_check=True)
                # un-scale the x256 preamble on ScalarE (VectorE stays free)
                nc.scalar.activation(hidbT_sb[:], ps_pre[:], AF.Copy,
                                     scale=1.0 / WSCALE)

            # ---- shared applied-attention accumulators --------------------
            appT_sb = consts.tile([128, KH * BL], f32)
            appT_bf = consts.tile([128, KH * BL], bf16)

            def emit_scores_mm(psc, ft, b):
                # psc (all 8 rows identical) += w2(ft) . th(ft,b)
                nc.tensor.matmul(
                    psc[:],
                    w2rep_sb[:, ft * BL:(ft + 1) * BL],
                    th_tiles[(ft, b)][:],
                    start=(ft == 0), stop=(ft == KF - 1))
                del th_tiles[(ft, b)]

            def emit_softmax_apply(psc, b):
                negmax = small_pool.tile([BL, 1], f32, tag="negmax",
                                         name="negmax")
                nc.vector.reduce_max(negmax[:], psc[:], axis=AX.X,
                                     negate=True)
                attn = attn_pool.tile([BL, S], bf16, tag="attn", name="attn")
                sumexp = small_pool.tile([BL, 1], f32, tag="sumexp",
                                         name="sumexp")
                nc.scalar.activation(attn[:], psc[:], AF.Exp,
                                     bias=negmax[:], scale=1.0,
                                     accum_out=sumexp[:])
                recip = small_pool.tile([BL, 1], f32, tag="recip",
                                        name="recip")
                nc.vector.reciprocal(recip[:], sumexp[:])
                # only row 0 is broadcast -- normalize just that row
                nc.vector.tensor_scalar_mul(attn[0:1, :], attn[0:1, :],
                                            recip[0:1, :])

                # broadcast attn row across 128 partitions via DRAM bounce
                attn_dr = dram_pool.tile([1, S], bf16, tag="attn_dr",
                                         name="attn_dr")
                nc.sync.dma_start(attn_dr[:], attn[0:1, :])
                abc = abc_pool.tile([128, S], bf16, tag="abc", name="abc")
                nc.sync.dma_start(abc[:],
                                  attn_dr[0:1, :].to_broadcast((128, S)))

                et = et_bf[b]
                for kc in range(0, KH):
                    scr = scr_pool.tile([128, S], bf16, tag="scr", name="scr")
                    nc.vector.tensor_tensor(
                        out=scr[:], in0=et[:, kc * S:(kc + 1) * S],
                        in1=abc[:], op=ALU.mult)
                    nc.vector.reduce_sum(
                        appT_sb[:, kc * BL + b: kc * BL + b + 1],
                        scr[:], axis=AX.X)
                nc.vector.tensor_copy(
                    appT_bf.rearrange("p (k b) -> p k b", b=BL)[:, :, b],
                    appT_sb.rearrange("p (k b) -> p k b", b=BL)[:, :, b])

            # ---- main sweep: NG groups of GB batch rows -------------------
            psT_pool = ctx.enter_context(
                tc.tile_pool(name="psT", bufs=4, space="PSUM"))
            psSc_pool = ctx.enter_context(
                tc.tile_pool(name="psSc", bufs=3, space="PSUM"))

            th_tiles = {}
            w2t_3d = w2t_sb.rearrange("p (k f) -> p k f", f=F)
            pending = []          # (ft, b) scores matmuls delayed one ft
            ps_out = None

            for g in range(NG):
                bs = range(g * GB, (g + 1) * GB)
                psc = {b: psSc_pool.tile([BL, S], f32, tag="psc",
                                         name=f"psc{b%GB}") for b in bs}
                psT = {}
                for ft in range(KF):
                    for kp in range(KH // 2):
                        # DoubleRow pair: contraction chunks 2kp, 2kp+1
                        lhs = w2t_3d[:, 2 * kp:2 * kp + 2,
                                     ft * 128:(ft + 1) * 128]
                        for b in bs:
                            if kp == 0:
                                psT[b] = psT_pool.tile(
                                    [128, S], f32, tag="pT", name=f"pT{b%GB}")
                            eq3 = et_q[b].rearrange("p (k s) -> p k s", s=S)
                            nc.tensor.matmul(
                                psT[b][:], lhs,
                                eq3[:, 2 * kp:2 * kp + 2, :],
                                start=(kp == 0), stop=(kp == KH // 2 - 1),
                                perf_mode=DR)
                    for b in bs:
                        t = th_pool.tile([128, S], bf16, tag="tanh",
                                         name="tanh")
                        nc.scalar.activation(
                            t[:], psT[b][:], AF.Tanh,
                            bias=hidbT_sb[:, ft * BL + b: ft * BL + b + 1],
                            scale=1.0 / WSCALE)
                        th_tiles[(ft, b)] = t
                    # scores run one ft behind so tanh is off the PE path
                    for (pft, pb) in pending:
                        emit_scores_mm(psc[pb], pft, pb)
                    pending = [(ft, b) for b in bs]

                    if ft == 0:
                        # non-critical DMAs ride behind the critical set:
                        # this group's bf16 enc (needed at group end) and
                        # the group-after-next's fp8 enc.
                        for b in bs:
                            et_bf[b] = load_encbf(b)
                        for b in range((g + 2) * GB,
                                       min((g + 3) * GB, BL)):
                            et_q[b] = load_encq(b)
                    if g == 2 and ft == 4:
                        # final combine, decoder_out half: out^T[f,b]
                        # accumulates in one PSUM bank with 8 sub-bank
                        # slices (single-start rule, see ps_pre).
                        psOut_pool = ctx.enter_context(
                            tc.tile_pool(name="psOut", bufs=1, space="PSUM"))
                        ps_out = psOut_pool.tile([128, 8 * BL], f32,
                                                 tag="out", name="psout")
                        for kc in range(KH):
                            w = wct_pool.tile([128, H], bf16, tag="wct",
                                              name="wctt")
                            nc.sync.dma_start(
                                w[:], wct[kc * 128:(kc + 1) * 128, :])
                            for fc in range(8):
                                nc.tensor.matmul(
                                    ps_out[:, fc * BL:(fc + 1) * BL],
                                    w[:, fc * 128:(fc + 1) * 128],
                                    decT_sb[:, kc * BL:(kc + 1) * BL],
                                    start=(kc == 0 and fc == 0), stop=False,
                                    skip_group_check=True)

                # last ft's scores, then softmax/apply for this group
                # (the V/S/DMA chain overlaps the next group's PE sweep)
                for (pft, pb) in pending:
                    emit_scores_mm(psc[pb], pft, pb)
                pending = []
                for b in bs:
                    emit_softmax_apply(psc[b], b)

            # ---- final combine, applied half + bias + tanh ----------------
            for kc in range(KH):
                w = wct_pool.tile([128, H], bf16, tag="wct", name="wctt")
                nc.sync.dma_start(w[:], wct[(KH + kc) * 128:
                                            (KH + kc + 1) * 128, :])
                for fc in range(8):
                    nc.tensor.matmul(
                        ps_out[:, fc * BL:(fc + 1) * BL],
                        w[:, fc * 128:(fc + 1) * 128],
                        appT_bf[:, kc * BL:(kc + 1) * BL],
                        start=False, stop=False,
                        skip_group_check=True)
            for fc in range(8):
                nc.tensor.matmul(
                    ps_out[:, fc * BL:(fc + 1) * BL],
                    b_comb_sb[:, fc * 128:(fc + 1) * 128],
                    ones8[:],
                    start=False, stop=(fc == 7),
                    skip_group_check=True)

            outT_sb = consts.tile([128, 8 * BL], f32)
            nc.scalar.activation(outT_sb[:], ps_out[:], AF.Tanh)
            for fc in range(8):
                nc.sync.dma_start(outT_d[fc * 128:(fc + 1) * 128, :],
                                  outT_sb[:, fc * BL:(fc + 1) * BL])
            for kc in range(KH):
                nc.sync.dma_start(appT_d[kc * 128:(kc + 1) * 128, :],
                                  appT_sb[:, kc * BL:(kc + 1) * BL])

    nc.compile()
    return nc


def _get_nc():
    if "nc" not in _CACHE:
        _CACHE["nc"] = _build()
    return _CACHE["nc"]


def make_in_maps(inputs):
    import ml_dtypes
    bf = ml_dtypes.bfloat16
    f8 = ml_dtypes.float8_e4m3fn

    inp = {k: np.asarray(v, dtype=np.float32) for k, v in inputs.items()}
    hidden = inp["hidden"]
    decoder_out = inp["decoder_out"]
    encoder_states = inp["encoder_states"]
    W_attn = inp["W_attn"]
    b_attn = inp["b_attn"]
    W_attn2 = inp["W_attn2"]
    W_comb = inp["W_comb"]
    b_comb = inp["b_comb"]
    # b_attn2 shifts every score equally -> softmax-invariant, unused.

    watT = np.ascontiguousarray(W_attn.T)                     # [2H, 2H]
    w1h = np.ascontiguousarray(watT[:H] * WSCALE).astype(f8)  # hidden-half
    w2t = np.ascontiguousarray(watT[H:] * WSCALE).astype(f8)  # encoder-half
    wct = np.ascontiguousarray(W_comb.T).astype(bf)
    w2rep = np.ascontiguousarray(np.repeat(W_attn2.reshape(F, 1), BL, axis=1))
    b_attn_2d = np.ascontiguousarray(b_attn.reshape(1, F) * WSCALE)
    b_comb_2d = np.ascontiguousarray(b_comb.reshape(1, H))

    in_maps = []
    for c in range(NCORES):
        sl = slice(c * BL, (c + 1) * BL)
        enc_c = np.ascontiguousarray(
            encoder_states[:, sl, :].transpose(1, 2, 0))      # [BL, H, S]
        enc_q = np.ascontiguousarray(
            enc_c.reshape(BL, KH, 128, S).transpose(0, 2, 1, 3)
            .reshape(BL, 128, KH * S)).astype(f8)             # [BL, p, k*s]
        in_maps.append({
            "enc_t": enc_c.astype(bf),
            "enc_q": enc_q,
            "w1h": w1h,
            "w2t": w2t,
            "wct": wct,
            "hidT": np.ascontiguousarray(hidden[sl].T),
            "decT": np.ascontiguousarray(decoder_out[sl].T),
            "w2rep": w2rep,
            "b_attn": b_attn_2d,
            "b_comb": b_comb_2d,
        })
    return in_maps


def kernel(**inputs):
    from concourse.bass_utils import run_bass_kernel_spmd

    in_maps = make_in_maps(inputs)
    nc = _get_nc()
    res = run_bass_kernel_spmd(nc, in_maps, list(range(NCORES)))
    out = np.concatenate(
        [np.ascontiguousarray(res.results[c]["outT"].T)
         for c in range(NCORES)], axis=0)
    applied = np.concatenate(
        [np.ascontiguousarray(res.results[c]["appliedT"].T)
         for c in range(NCORES)], axis=0)
    return out.astype(np.float32), applied.astype(np.float32)


# revision 28
# speedup vs baseline: 1.5339x; 1.1077x over previous
"""Trainium2 Bass kernel for nn_AttentionModule (Bahdanau-style attention).

Reference computation (S=512, B=64, H=1024, F=2H):
    cat    = concat([hidden bcast to (S,B,H), encoder_states], -1)      [S,B,2H]
    scores = tanh(cat @ W_attn.T + b_attn) @ W_attn2.T + b_attn2        [S,B,1]
    attn   = softmax(scores[..., 0].T, axis=-1)                         [B,S]
    applied= einsum("bs,sbh->bh", attn, encoder_states)                 [B,H]
    out    = tanh(concat([decoder_out, applied], -1) @ W_comb.T + b_comb)

Sharding: data-parallel over B across 8 cores (8 batch rows per core).

Optimized structure (vs the bf16 baseline):
  - The dominant matmul T[f,s] = W1e @ enc (per b: [2048,1024]@[1024,512])
    runs in fp8e4 with perf_mode=DoubleRow: each instruction consumes two
    128-deep contraction chunks at once (~1.8x the bf16 rate).  W1e and
    W1h ship pre-scaled by 256 (fp8e4 subnormal range); consumers
    un-scale with activation scale=1/256.  The encoder ships twice: bf16
    [b,(kc p),s] for the attention apply, and fp8 pre-arranged to
    [b,p,(kc s)] (plain 2D DMA, 4KB rows) for the matmul.
  - Batch rows are processed in groups of 2 with b innermost so each
    DoubleRow weight load is reused (LDWEIGHTS has no FWL in DoubleRow
    mode).  PSUM budget (8 banks): psPre(1) closes after the preamble,
    then psT(4) + psSc(3: per-b score accumulator banks) + psOut(1).
  - The scores matmul (contract f: w2 . tanh) accumulates inline, one
    partial matmul per (ft,b) emitted one ft behind the main sweep.
  - hid @ W1h.T preamble and the final combine run operand-swapped
    (weights stationary, batch as the 8-wide moving operand) producing
    transposed outputs directly -- no PE transposes.  The decoder_out
    half of the final combine is emitted mid-sweep, off the critical
    tail.
  - DMA issue costs ~0.6us per descriptor on the issuing engine's queue,
    so transfers are batched (small constants concatenated host-side
    into one tensor) and split across the two HWDGE queues: critical
    loads (consts, w1h, w2t, enc_q) on SyncE, bulk/late loads (enc bf16,
    wct, attn bounces) on ScalarE.
  - softmax -> DRAM-bounce broadcast -> apply (VectorE mult+reduce over
    resident bf16 enc tiles), overlapping the next group's PE sweep.

Known pitfalls baked in:
  - small tensors ship fp32 and are cast on device (tiny bf16 rows get
    corrupted host->device); >=32B inner blocks for rearrange DMAs.
  - 16/32-bit matmul operand mixing rejected; fp8 pairs must both be fp8.
  - PSUM zero regions are 2KB: one accumulation group per bank region;
    sub-bank slices may emit start=True only on first touch.
  - vector.tensor_tensor_reduce breaks hardware execution (INTERNAL
    error) though CoreSim accepts it -- use tensor_tensor + reduce_sum.
"""

import numpy as np

S, B, H = 512, 64, 1024
F = 2 * H
NCORES = 8
BL = B // NCORES          # 8 batch rows per core
KH = H // 128             # 8 contraction chunks over H
KF = F // 128             # 16 feature tiles
GB = 2                    # batch rows per group (PSUM-bank limited)
NG = BL // GB             # 4 groups
WSCALE = 256.0            # fp8 weight pre-scale (power of 2)

_CACHE = {}


def _build(num_devices=NCORES):
    from contextlib import ExitStack

    import concourse.tile as tile
    from concourse import bacc, mybir

    f32 = mybir.dt.float32
    bf16 = mybir.dt.bfloat16
    fp8 = mybir.dt.float8e4
    AF = mybir.ActivationFunctionType
    ALU = mybir.AluOpType
    AX = mybir.AxisListType
    DR = mybir.MatmulPerfMode.DoubleRow

    nc = bacc.Bacc("TRN2", target_bir_lowering=False, debug=False,
                   num_devices=num_devices)

    enc_t = nc.dram_tensor("enc_t", [BL, H, S], bf16, kind="ExternalInput").ap()
    # fp8 encoder copy, host-pre-arranged to [b, partition, kc*S+s] so the
    # load is a plain 2D DMA with 4KB-contiguous rows per partition.
    enc_q_d = nc.dram_tensor("enc_q", [BL, 128, KH * S], fp8,
                             kind="ExternalInput").ap()
    w1h = nc.dram_tensor("w1h", [H, F], fp8, kind="ExternalInput").ap()
    w2t = nc.dram_tensor("w2t", [H, F], fp8, kind="ExternalInput").ap()
    wct = nc.dram_tensor("wct", [F, H], bf16, kind="ExternalInput").ap()
    # smalls = concat([hidden.T, decoder_out.T, W_attn2.T x8], axis=0)
    smalls_d = nc.dram_tensor("smalls", [4 * H, BL], f32,
                              kind="ExternalInput").ap()
    # bias = concat([b_attn * 256, b_comb], axis=1)
    bias_d = nc.dram_tensor("bias", [1, F + H], f32, kind="ExternalInput").ap()
    outT_d = nc.dram_tensor("outT", [H, BL], f32, kind="ExternalOutput").ap()
    appT_d = nc.dram_tensor("appliedT", [H, BL], f32,
                            kind="ExternalOutput").ap()

    with tile.TileContext(nc) as tc:
        with ExitStack() as ctx:
            consts = ctx.enter_context(tc.tile_pool(name="consts", bufs=1))
            encbf_pool = ctx.enter_context(tc.tile_pool(name="encbf", bufs=BL))
            encq_pool = ctx.enter_context(tc.tile_pool(name="encq", bufs=BL))
            w1h_pool = ctx.enter_context(tc.tile_pool(name="w1h", bufs=8))
            th_pool = ctx.enter_context(tc.tile_pool(name="th", bufs=10))
            attn_pool = ctx.enter_context(tc.tile_pool(name="attn", bufs=2))
            abc_pool = ctx.enter_context(tc.tile_pool(name="abc", bufs=2))
            scr_pool = ctx.enter_context(tc.tile_pool(name="scr", bufs=2))
            small_pool = ctx.enter_context(tc.tile_pool(name="small", bufs=8))
            dram_pool = ctx.enter_context(
                tc.tile_pool(name="dram", bufs=2, space="DRAM"))

            # ---- batched constants (one DMA each, SyncE queue) ------------
            ones8 = consts.tile([1, BL], bf16)
            nc.vector.memset(ones8[:], 1.0)
            smalls_32 = consts.tile([128, 32 * BL], f32)
            nc.sync.dma_start(
                smalls_32.rearrange("p (k b) -> p k b", b=BL),
                smalls_d.rearrange("(k p) b -> p k b", p=128))
            bias_32 = consts.tile([1, F + H], f32)
            nc.sync.dma_start(bias_32[:], bias_d[:])

            hidT_q = consts.tile([128, KH * BL], fp8)
            nc.vector.tensor_copy(hidT_q[:], smalls_32[:, 0:KH * BL])
            decT_sb = consts.tile([128, KH * BL], bf16)
            nc.vector.tensor_copy(decT_sb[:],
                                  smalls_32[:, KH * BL:2 * KH * BL])
            w2rep_sb = consts.tile([128, KF * BL], bf16)
            nc.vector.tensor_copy(w2rep_sb[:],
                                  smalls_32[:, 2 * KH * BL:4 * KH * BL])
            b_attn_sb = consts.tile([1, F], bf16)
            nc.vector.tensor_copy(b_attn_sb[:], bias_32[:, 0:F])
            b_comb_sb = consts.tile([1, H], bf16)
            nc.vector.tensor_copy(b_comb_sb[:], bias_32[:, F:])

            # ---- critical-path loads (SyncE): w1h, w2t, first fp8 enc -----
            w1h_tiles = []
            for kc in range(KH):
                w1c = w1h_pool.tile([128, F], fp8, tag="w1h", name="w1c")
                nc.sync.dma_start(w1c[:], w1h[kc * 128:(kc + 1) * 128, :])
                w1h_tiles.append(w1c)

            w2t_sb = consts.tile([128, KH * F], fp8)
            for kc in range(KH):
                nc.sync.dma_start(w2t_sb[:, kc * F:(kc + 1) * F],
                                  w2t[kc * 128:(kc + 1) * 128, :])

            def load_encq(b):
                t = encq_pool.tile([128, KH * S], fp8, tag="encq",
                                   name=f"encq{b}")
                nc.sync.dma_start(t[:], enc_q_d[b])
                return t

            def load_encbf(b):
                # bulk load on the ScalarE HWDGE queue
                t = encbf_pool.tile([128, KH * S], bf16, tag="encbf",
                                    name=f"encbf{b}")
                nc.scalar.dma_start(
                    t.rearrange("p (k s) -> p k s", s=S),
                    enc_t[b].rearrange("(k p) s -> p k s", p=128))
                return t

            et_q = {b: load_encq(b) for b in range(2 * GB)}
            et_bf = {}

            # ---- preamble: hidbT[f, b] = (hidden @ W1h.T + b_attn)^T ------
            # operand-swapped: W1h chunks stationary (fp8, x256), batch (8)
            # moving; output lands transposed (f on partitions), un-scaled
            # by the copy-out.  ps_pre is one PSUM bank with 16 sub-bank
            # slices: start=True only on first touch (2KB zero region).
            hidbT_sb = consts.tile([128, KF * BL], f32)
            with tc.tile_pool(name="psPre", bufs=1, space="PSUM") as psPre_pool:
                ps_pre = psPre_pool.tile([128, KF * BL], f32, tag="pre",
                                         name="pspre")
                for kc in range(KH):
                    w1c = w1h_tiles[kc]
                    for ft in range(KF):
                        nc.tensor.matmul(
                            ps_pre[:, ft * BL:(ft + 1) * BL],
                            w1c[:, ft * 128:(ft + 1) * 128],
                            hidT_q[:, kc * BL:(kc + 1) * BL],
                            start=(kc == 0 and ft == 0), stop=False,
                            skip_group_check=True)
                for ft in range(KF):
                    nc.tensor.matmul(
                        ps_pre[:, ft * BL:(ft + 1) * BL],
                        b_attn_sb[:, ft * 128:(ft + 1) * 128],
                        ones8[:],
                        start=False, stop=(ft == KF - 1),
                        skip_group_check=True)
                # un-scale the x256 preamble on ScalarE (VectorE stays free)
                nc.scalar.activation(hidbT_sb[:], ps_pre[:], AF.Copy,
                                     scale=1.0 / WSCALE)

            # ---- shared applied-attention accumulators --------------------
            appT_sb = consts.tile([128, KH * BL], f32)
            appT_bf = consts.tile([128, KH * BL], bf16)

            def emit_scores_mm(psc, ft, b):
                # psc (all 8 rows identical) += w2(ft) . th(ft,b)
                nc.tensor.matmul(
                    psc[:],
                    w2rep_sb[:, ft * BL:(ft + 1) * BL],
                    th_tiles[(ft, b)][:],
                    start=(ft == 0), stop=(ft == KF - 1))
                del th_tiles[(ft, b)]

            def emit_softmax_apply(psc, b):
                negmax = small_pool.tile([BL, 1], f32, tag="negmax",
                                         name="negmax")
                nc.vector.reduce_max(negmax[:], psc[:], axis=AX.X,
                                     negate=True)
                attn = attn_pool.tile([BL, S], bf16, tag="attn", name="attn")
                sumexp = small_pool.tile([BL, 1], f32, tag="sumexp",
                                         name="sumexp")
                nc.scalar.activation(attn[:], psc[:], AF.Exp,
                                     bias=negmax[:], scale=1.0,
                                     accum_out=sumexp[:])
                recip = small_pool.tile([BL, 1], f32, tag="recip",
                                        name="recip")
                nc.vector.reciprocal(recip[:], sumexp[:])
                # only row 0 is broadcast -- normalize just that row
                nc.vector.tensor_scalar_mul(attn[0:1, :], attn[0:1, :],
                                            recip[0:1, :])

                # broadcast attn row across 128 partitions via DRAM bounce
                attn_dr = dram_pool.tile([1, S], bf16, tag="attn_dr",
                                         name="attn_dr")
                nc.scalar.dma_start(attn_dr[:], attn[0:1, :])
                abc = abc_pool.tile([128, S], bf16, tag="abc", name="abc")
                nc.scalar.dma_start(abc[:],
                                    attn_dr[0:1, :].to_broadcast((128, S)))

                et = et_bf[b]
                for kc in range(KH):
                    scr = scr_pool.tile([128, S], bf16, tag="scr", name="scr")
                    nc.vector.tensor_tensor(
                        out=scr[:], in0=et[:, kc * S:(kc + 1) * S],
                        in1=abc[:], op=ALU.mult)
                    nc.vector.reduce_sum(
                        appT_sb[:, kc * BL + b: kc * BL + b + 1],
                        scr[:], axis=AX.X)
                nc.vector.tensor_copy(
                    appT_bf.rearrange("p (k b) -> p k b", b=BL)[:, :, b],
                    appT_sb.rearrange("p (k b) -> p k b", b=BL)[:, :, b])

            # ---- main sweep: NG groups of GB batch rows -------------------
            psT_pool = ctx.enter_context(
                tc.tile_pool(name="psT", bufs=4, space="PSUM"))
            psSc_pool = ctx.enter_context(
                tc.tile_pool(name="psSc", bufs=3, space="PSUM"))

            th_tiles = {}
            w2t_3d = w2t_sb.rearrange("p (k f) -> p k f", f=F)
            pending = []          # (ft, b) scores matmuls delayed one ft
            ps_out = None
            wct_dec = wct_app = None

            for g in range(NG):
                bs = range(g * GB, (g + 1) * GB)
                psc = {b: psSc_pool.tile([BL, S], f32, tag="psc",
                                         name=f"psc{b%GB}") for b in bs}
                psT = {}
                for ft in range(KF):
                    for kp in range(KH // 2):
                        # DoubleRow pair: contraction chunks 2kp, 2kp+1
                        lhs = w2t_3d[:, 2 * kp:2 * kp + 2,
                                     ft * 128:(ft + 1) * 128]
                        for b in bs:
                            if kp == 0:
                                psT[b] = psT_pool.tile(
                                    [128, S], f32, tag="pT", name=f"pT{b%GB}")
                            eq3 = et_q[b].rearrange("p (k s) -> p k s", s=S)
                            nc.tensor.matmul(
                                psT[b][:], lhs,
                                eq3[:, 2 * kp:2 * kp + 2, :],
                                start=(kp == 0), stop=(kp == KH // 2 - 1),
                                perf_mode=DR)
                    for b in bs:
                        t = th_pool.tile([128, S], bf16, tag="tanh",
                                         name="tanh")
                        nc.scalar.activation(
                            t[:], psT[b][:], AF.Tanh,
                            bias=hidbT_sb[:, ft * BL + b: ft * BL + b + 1],
                            scale=1.0 / WSCALE)
                        th_tiles[(ft, b)] = t
                    # scores run one ft behind so tanh is off the PE path
                    for (pft, pb) in pending:
                        emit_scores_mm(psc[pb], pft, pb)
                    pending = [(ft, b) for b in bs]

                    if ft == 0:
                        # non-critical DMAs ride behind the critical set:
                        # this group's bf16 enc (needed at group end) and
                        # the group-after-next's fp8 enc.
                        for b in bs:
                            et_bf[b] = load_encbf(b)
                        for b in range((g + 2) * GB,
                                       min((g + 3) * GB, BL)):
                            et_q[b] = load_encq(b)
                    if g == 1 and ft == 8:
                        # wct decoder-half block load (ScalarE queue)
                        wct_dec = consts.tile([128, KH * H], bf16)
                        nc.scalar.dma_start(
                            wct_dec.rearrange("p (k h) -> p k h", h=H),
                            wct[0:H].rearrange("(k p) h -> p k h", p=128))
                    if g == 2 and ft == 4:
                        # final combine, decoder_out half: out^T[f,b]
                        # accumulates in one PSUM bank with 8 sub-bank
                        # slices (single-start rule, see ps_pre).
                        psOut_pool = ctx.enter_context(
                            tc.tile_pool(name="psOut", bufs=1, space="PSUM"))
                        ps_out = psOut_pool.tile([128, 8 * BL], f32,
                                                 tag="out", name="psout")
                        for kc in range(KH):
                            for fc in range(8):
                                nc.tensor.matmul(
                                    ps_out[:, fc * BL:(fc + 1) * BL],
                                    wct_dec[:, kc * H + fc * 128:
                                            kc * H + (fc + 1) * 128],
                                    decT_sb[:, kc * BL:(kc + 1) * BL],
                                    start=(kc == 0 and fc == 0), stop=False,
                                    skip_group_check=True)
                    if g == 2 and ft == 8:
                        # wct applied-half block load (ScalarE queue)
                        wct_app = consts.tile([128, KH * H], bf16)
                        nc.scalar.dma_start(
                            wct_app.rearrange("p (k h) -> p k h", h=H),
                            wct[H:F].rearrange("(k p) h -> p k h", p=128))

                # last ft's scores, then softmax/apply for this group
                # (the V/S/DMA chain overlaps the next group's PE sweep)
                for (pft, pb) in pending:
                    emit_scores_mm(psc[pb], pft, pb)
                pending = []
                for b in bs:
                    emit_softmax_apply(psc[b], b)

            # ---- final combine, applied half + bias + tanh ----------------
            for kc in range(KH):
                for fc in range(8):
                    nc.tensor.matmul(
                        ps_out[:, fc * BL:(fc + 1) * BL],
                        wct_app[:, kc * H + fc * 128:
                                kc * H + (fc + 1) * 128],
                        appT_bf[:, kc * BL:(kc + 1) * BL],
                        start=False, stop=False,
                        skip_group_check=True)
            for fc in range(8):
                nc.tensor.matmul(
                    ps_out[:, fc * BL:(fc + 1) * BL],
                    b_comb_sb[:, fc * 128:(fc + 1) * 128],
                    ones8[:],
                    start=False, stop=(fc == 7),
                    skip_group_check=True)

            outT_sb = consts.tile([128, 8 * BL], f32)
            nc.scalar.activation(outT_sb[:], ps_out[:], AF.Tanh)
            nc.sync.dma_start(
                outT_d.rearrange("(k p) b -> p k b", p=128),
                outT_sb.rearrange("p (k b) -> p k b", b=BL))
            nc.sync.dma_start(
                appT_d.rearrange("(k p) b -> p k b", p=128),
                appT_sb.rearrange("p (k b) -> p k b", b=BL))

    nc.compile()
    return nc


def _get_nc():
    if "nc" not in _CACHE:
        _CACHE["nc"] = _build()
    return _CACHE["nc"]


def make_in_maps(inputs):
    import ml_dtypes
    bf = ml_dtypes.bfloat16
    f8 = ml_dtypes.float8_e4m3fn

    inp = {k: np.asarray(v, dtype=np.float32) for k, v in inputs.items()}
    hidden = inp["hidden"]
    decoder_out = inp["decoder_out"]
    encoder_states = inp["encoder_states"]
    W_attn = inp["W_attn"]
    b_attn = inp["b_attn"]
    W_attn2 = inp["W_attn2"]
    W_comb = inp["W_comb"]
    b_comb = inp["b_comb"]
    # b_attn2 shifts every score equally -> softmax-invariant, unused.

    watT = np.ascontiguousarray(W_attn.T)                     # [2H, 2H]
    w1h = np.ascontiguousarray(watT[:H] * WSCALE).astype(f8)  # hidden-half
    w2t = np.ascontiguousarray(watT[H:] * WSCALE).astype(f8)  # encoder-half
    wct = np.ascontiguousarray(W_comb.T).astype(bf)
    w2rep = np.repeat(W_attn2.reshape(F, 1), BL, axis=1)
    bias = np.ascontiguousarray(np.concatenate(
        [b_attn.reshape(1, F) * WSCALE, b_comb.reshape(1, H)],
        axis=1).astype(np.float32))

    in_maps = []
    for c in range(NCORES):
        sl = slice(c * BL, (c + 1) * BL)
        enc_c = np.ascontiguousarray(
            encoder_states[:, sl, :].transpose(1, 2, 0))      # [BL, H, S]
        enc_q = np.ascontiguousarray(
            enc_c.reshape(BL, KH, 128, S).transpose(0, 2, 1, 3)
            .reshape(BL, 128, KH * S)).astype(f8)             # [BL, p, k*s]
        smalls = np.ascontiguousarray(np.concatenate(
            [hidden[sl].T, decoder_out[sl].T, w2rep],
            axis=0).astype(np.float32))                       # [4H, BL]
        in_maps.append({
            "enc_t": enc_c.astype(bf),
            "enc_q": enc_q,
            "w1h": w1h,
            "w2t": w2t,
            "wct": wct,
            "smalls": smalls,
            "bias": bias,
        })
    return in_maps


def kernel(**inputs):
    from concourse.bass_utils import run_bass_kernel_spmd

    in_maps = make_in_maps(inputs)
    nc = _get_nc()
    res = run_bass_kernel_spmd(nc, in_maps, list(range(NCORES)))
    out = np.concatenate(
        [np.ascontiguousarray(res.results[c]["outT"].T)
         for c in range(NCORES)], axis=0)
    applied = np.concatenate(
        [np.ascontiguousarray(res.results[c]["appliedT"].T)
         for c in range(NCORES)], axis=0)
    return out.astype(np.float32), applied.astype(np.float32)


# revision 29
# speedup vs baseline: 1.5529x; 1.0124x over previous
"""Trainium2 Bass kernel for nn_AttentionModule (Bahdanau-style attention).

Reference computation (S=512, B=64, H=1024, F=2H):
    cat    = concat([hidden bcast to (S,B,H), encoder_states], -1)      [S,B,2H]
    scores = tanh(cat @ W_attn.T + b_attn) @ W_attn2.T + b_attn2        [S,B,1]
    attn   = softmax(scores[..., 0].T, axis=-1)                         [B,S]
    applied= einsum("bs,sbh->bh", attn, encoder_states)                 [B,H]
    out    = tanh(concat([decoder_out, applied], -1) @ W_comb.T + b_comb)

Sharding: data-parallel over B across 8 cores (8 batch rows per core).

Optimized structure (vs the bf16 baseline):
  - The dominant matmul T[f,s] = W1e @ enc (per b: [2048,1024]@[1024,512])
    runs in fp8e4 with perf_mode=DoubleRow: each instruction consumes two
    128-deep contraction chunks at once (~1.8x the bf16 rate).  W1e and
    W1h ship pre-scaled by 256 (fp8e4 subnormal range); consumers
    un-scale with activation scale=1/256.  The encoder ships twice: bf16
    [b,(kc p),s] for the attention apply, and fp8 pre-arranged to
    [b,p,(kc s)] (plain 2D DMA, 4KB rows) for the matmul.
  - Batch rows are processed in groups of 2 with b innermost so each
    DoubleRow weight load is reused (LDWEIGHTS has no FWL in DoubleRow
    mode).  PSUM budget (8 banks): psPre(1) closes after the preamble,
    then psT(4) + psSc(3: per-b score accumulator banks) + psOut(1).
  - The scores matmul (contract f: w2 . tanh) accumulates inline, one
    partial matmul per (ft,b) emitted one ft behind the main sweep.
  - hid @ W1h.T preamble and the final combine run operand-swapped
    (weights stationary, batch as the 8-wide moving operand) producing
    transposed outputs directly -- no PE transposes.  The decoder_out
    half of the final combine is emitted mid-sweep, off the critical
    tail.
  - DMA issue costs ~0.6us per descriptor on the issuing engine's queue,
    so transfers are batched (small constants concatenated host-side
    into one tensor) and split across the two HWDGE queues: critical
    loads (consts, w1h, w2t, enc_q) on SyncE, bulk/late loads (enc bf16,
    wct, attn bounces) on ScalarE.
  - softmax -> DRAM-bounce broadcast -> apply (VectorE mult+reduce over
    resident bf16 enc tiles), overlapping the next group's PE sweep.

Known pitfalls baked in:
  - small tensors ship fp32 and are cast on device (tiny bf16 rows get
    corrupted host->device); >=32B inner blocks for rearrange DMAs.
  - 16/32-bit matmul operand mixing rejected; fp8 pairs must both be fp8.
  - PSUM zero regions are 2KB: one accumulation group per bank region;
    sub-bank slices may emit start=True only on first touch.
  - vector.tensor_tensor_reduce breaks hardware execution (INTERNAL
    error) though CoreSim accepts it -- use tensor_tensor + reduce_sum.
"""

import numpy as np

S, B, H = 512, 64, 1024
F = 2 * H
NCORES = 8
BL = B // NCORES          # 8 batch rows per core
KH = H // 128             # 8 contraction chunks over H
KF = F // 128             # 16 feature tiles
GB = 2                    # batch rows per group (PSUM-bank limited)
NG = BL // GB             # 4 groups
WSCALE = 256.0            # fp8 weight pre-scale (power of 2)

_CACHE = {}


def _build(num_devices=NCORES):
    from contextlib import ExitStack

    import concourse.tile as tile
    from concourse import bacc, mybir

    f32 = mybir.dt.float32
    bf16 = mybir.dt.bfloat16
    fp8 = mybir.dt.float8e4
    AF = mybir.ActivationFunctionType
    ALU = mybir.AluOpType
    AX = mybir.AxisListType
    DR = mybir.MatmulPerfMode.DoubleRow

    nc = bacc.Bacc("TRN2", target_bir_lowering=False, debug=False,
                   num_devices=num_devices)

    # encoder copies, host-pre-arranged to [b, partition, kc*S+s] so the
    # loads are plain 2D DMAs with contiguous rows per partition (strided
    # rearrange DMAs cost ~us-scale descriptor generation on the queue).
    enc_t = nc.dram_tensor("enc_t", [BL, 128, KH * S], bf16,
                           kind="ExternalInput").ap()
    enc_q_d = nc.dram_tensor("enc_q", [BL, 128, KH * S], fp8,
                             kind="ExternalInput").ap()
    w1h = nc.dram_tensor("w1h", [H, F], fp8, kind="ExternalInput").ap()
    w2t = nc.dram_tensor("w2t", [H, F], fp8, kind="ExternalInput").ap()
    # wct halves pre-arranged to [half, partition, kc*H+h]
    wct = nc.dram_tensor("wct", [2, 128, KH * H], bf16,
                         kind="ExternalInput").ap()
    # smalls pre-arranged to [partition, k*BL+b] (hidT | decT | w2rep)
    smalls_d = nc.dram_tensor("smalls", [128, 32 * BL], f32,
                              kind="ExternalInput").ap()
    # bias = concat([b_attn * 256, b_comb], axis=1)
    bias_d = nc.dram_tensor("bias", [1, F + H], f32, kind="ExternalInput").ap()
    # outputs stay in the on-chip [partition, k*BL+b] layout; the host
    # un-arranges (keeps the tail DMAs to two fast contiguous writes).
    outT_d = nc.dram_tensor("outT", [128, 8 * BL], f32,
                            kind="ExternalOutput").ap()
    appT_d = nc.dram_tensor("appliedT", [128, KH * BL], f32,
                            kind="ExternalOutput").ap()

    with tile.TileContext(nc) as tc:
        with ExitStack() as ctx:
            consts = ctx.enter_context(tc.tile_pool(name="consts", bufs=1))
            encbf_pool = ctx.enter_context(tc.tile_pool(name="encbf", bufs=BL))
            encq_pool = ctx.enter_context(tc.tile_pool(name="encq", bufs=BL))
            w1h_pool = ctx.enter_context(tc.tile_pool(name="w1h", bufs=8))
            th_pool = ctx.enter_context(tc.tile_pool(name="th", bufs=10))
            attn_pool = ctx.enter_context(tc.tile_pool(name="attn", bufs=2))
            abc_pool = ctx.enter_context(tc.tile_pool(name="abc", bufs=2))
            scr_pool = ctx.enter_context(tc.tile_pool(name="scr", bufs=2))
            small_pool = ctx.enter_context(tc.tile_pool(name="small", bufs=8))
            dram_pool = ctx.enter_context(
                tc.tile_pool(name="dram", bufs=2, space="DRAM"))

            # ---- batched constants (one DMA each, SyncE queue) ------------
            ones8 = consts.tile([1, BL], bf16)
            nc.vector.memset(ones8[:], 1.0)
            smalls_32 = consts.tile([128, 32 * BL], f32)
            nc.sync.dma_start(smalls_32[:], smalls_d[:])
            bias_32 = consts.tile([1, F + H], f32)
            nc.sync.dma_start(bias_32[:], bias_d[:])

            hidT_q = consts.tile([128, KH * BL], fp8)
            nc.vector.tensor_copy(hidT_q[:], smalls_32[:, 0:KH * BL])
            decT_sb = consts.tile([128, KH * BL], bf16)
            nc.vector.tensor_copy(decT_sb[:],
                                  smalls_32[:, KH * BL:2 * KH * BL])
            w2rep_sb = consts.tile([128, KF * BL], bf16)
            nc.vector.tensor_copy(w2rep_sb[:],
                                  smalls_32[:, 2 * KH * BL:4 * KH * BL])
            b_attn_sb = consts.tile([1, F], bf16)
            nc.vector.tensor_copy(b_attn_sb[:], bias_32[:, 0:F])
            b_comb_sb = consts.tile([1, H], bf16)
            nc.vector.tensor_copy(b_comb_sb[:], bias_32[:, F:])

            # ---- critical-path loads (SyncE): w1h, w2t, first fp8 enc -----
            w1h_tiles = []
            for kc in range(KH):
                w1c = w1h_pool.tile([128, F], fp8, tag="w1h", name="w1c")
                nc.sync.dma_start(w1c[:], w1h[kc * 128:(kc + 1) * 128, :])
                w1h_tiles.append(w1c)

            w2t_sb = consts.tile([128, KH * F], fp8)
            for kc in range(KH):
                nc.sync.dma_start(w2t_sb[:, kc * F:(kc + 1) * F],
                                  w2t[kc * 128:(kc + 1) * 128, :])

            def load_encq(b):
                t = encq_pool.tile([128, KH * S], fp8, tag="encq",
                                   name=f"encq{b}")
                nc.sync.dma_start(t[:], enc_q_d[b])
                return t

            def load_encbf(b):
                # bulk load on the ScalarE HWDGE queue
                t = encbf_pool.tile([128, KH * S], bf16, tag="encbf",
                                    name=f"encbf{b}")
                nc.scalar.dma_start(t[:], enc_t[b])
                return t

            et_q = {b: load_encq(b) for b in range(2 * GB)}
            et_bf = {}

            # ---- preamble: hidbT[f, b] = (hidden @ W1h.T + b_attn)^T ------
            # operand-swapped: W1h chunks stationary (fp8, x256), batch (8)
            # moving; output lands transposed (f on partitions), un-scaled
            # by the copy-out.  ps_pre is one PSUM bank with 16 sub-bank
            # slices: start=True only on first touch (2KB zero region).
            hidbT_sb = consts.tile([128, KF * BL], f32)
            with tc.tile_pool(name="psPre", bufs=1, space="PSUM") as psPre_pool:
                ps_pre = psPre_pool.tile([128, KF * BL], f32, tag="pre",
                                         name="pspre")
                for kc in range(KH):
                    w1c = w1h_tiles[kc]
                    for ft in range(KF):
                        nc.tensor.matmul(
                            ps_pre[:, ft * BL:(ft + 1) * BL],
                            w1c[:, ft * 128:(ft + 1) * 128],
                            hidT_q[:, kc * BL:(kc + 1) * BL],
                            start=(kc == 0 and ft == 0), stop=False,
                            skip_group_check=True)
                for ft in range(KF):
                    nc.tensor.matmul(
                        ps_pre[:, ft * BL:(ft + 1) * BL],
                        b_attn_sb[:, ft * 128:(ft + 1) * 128],
                        ones8[:],
                        start=False, stop=(ft == KF - 1),
                        skip_group_check=True)
                # un-scale the x256 preamble on ScalarE (VectorE stays free)
                nc.scalar.activation(hidbT_sb[:], ps_pre[:], AF.Copy,
                                     scale=1.0 / WSCALE)

            # ---- shared applied-attention accumulators --------------------
            appT_sb = consts.tile([128, KH * BL], f32)
            appT_bf = consts.tile([128, KH * BL], bf16)

            def emit_scores_mm(psc, ft, b):
                # psc (all 8 rows identical) += w2(ft) . th(ft,b)
                nc.tensor.matmul(
                    psc[:],
                    w2rep_sb[:, ft * BL:(ft + 1) * BL],
                    th_tiles[(ft, b)][:],
                    start=(ft == 0), stop=(ft == KF - 1))
                del th_tiles[(ft, b)]

            def emit_softmax_apply(psc, b):
                negmax = small_pool.tile([BL, 1], f32, tag="negmax",
                                         name="negmax")
                nc.vector.reduce_max(negmax[:], psc[:], axis=AX.X,
                                     negate=True)
                attn = attn_pool.tile([BL, S], bf16, tag="attn", name="attn")
                sumexp = small_pool.tile([BL, 1], f32, tag="sumexp",
                                         name="sumexp")
                nc.scalar.activation(attn[:], psc[:], AF.Exp,
                                     bias=negmax[:], scale=1.0,
                                     accum_out=sumexp[:])
                recip = small_pool.tile([BL, 1], f32, tag="recip",
                                        name="recip")
                nc.vector.reciprocal(recip[:], sumexp[:])
                # only row 0 is broadcast -- normalize just that row
                nc.vector.tensor_scalar_mul(attn[0:1, :], attn[0:1, :],
                                            recip[0:1, :])

                # broadcast attn row across 128 partitions via DRAM bounce
                attn_dr = dram_pool.tile([1, S], bf16, tag="attn_dr",
                                         name="attn_dr")
                nc.scalar.dma_start(attn_dr[:], attn[0:1, :])
                abc = abc_pool.tile([128, S], bf16, tag="abc", name="abc")
                nc.scalar.dma_start(abc[:],
                                    attn_dr[0:1, :].to_broadcast((128, S)))

                et = et_bf[b]
                for kc in range(KH):
                    scr = scr_pool.tile([128, S], bf16, tag="scr", name="scr")
                    nc.vector.tensor_tensor(
                        out=scr[:], in0=et[:, kc * S:(kc + 1) * S],
                        in1=abc[:], op=ALU.mult)
                    nc.vector.reduce_sum(
                        appT_sb[:, kc * BL + b: kc * BL + b + 1],
                        scr[:], axis=AX.X)
                nc.vector.tensor_copy(
                    appT_bf.rearrange("p (k b) -> p k b", b=BL)[:, :, b],
                    appT_sb.rearrange("p (k b) -> p k b", b=BL)[:, :, b])

            # ---- main sweep: NG groups of GB batch rows -------------------
            psT_pool = ctx.enter_context(
                tc.tile_pool(name="psT", bufs=4, space="PSUM"))
            psSc_pool = ctx.enter_context(
                tc.tile_pool(name="psSc", bufs=3, space="PSUM"))

            th_tiles = {}
            w2t_3d = w2t_sb.rearrange("p (k f) -> p k f", f=F)
            pending = []          # (ft, b) scores matmuls delayed one ft
            ps_out = None
            wct_dec = wct_app = None

            for g in range(NG):
                bs = range(g * GB, (g + 1) * GB)
                psc = {b: psSc_pool.tile([BL, S], f32, tag="psc",
                                         name=f"psc{b%GB}") for b in bs}
                psT = {}
                for ft in range(KF):
                    for kp in range(KH // 2):
                        # DoubleRow pair: contraction chunks 2kp, 2kp+1
                        lhs = w2t_3d[:, 2 * kp:2 * kp + 2,
                                     ft * 128:(ft + 1) * 128]
                        for b in bs:
                            if kp == 0:
                                psT[b] = psT_pool.tile(
                                    [128, S], f32, tag="pT", name=f"pT{b%GB}")
                            eq3 = et_q[b].rearrange("p (k s) -> p k s", s=S)
                            nc.tensor.matmul(
                                psT[b][:], lhs,
                                eq3[:, 2 * kp:2 * kp + 2, :],
                                start=(kp == 0), stop=(kp == KH // 2 - 1),
                                perf_mode=DR)
                    for b in bs:
                        t = th_pool.tile([128, S], bf16, tag="tanh",
                                         name="tanh")
                        nc.scalar.activation(
                            t[:], psT[b][:], AF.Tanh,
                            bias=hidbT_sb[:, ft * BL + b: ft * BL + b + 1],
                            scale=1.0 / WSCALE)
                        th_tiles[(ft, b)] = t
                    # scores run one ft behind so tanh is off the PE path
                    for (pft, pb) in pending:
                        emit_scores_mm(psc[pb], pft, pb)
                    pending = [(ft, b) for b in bs]

                    if ft == 0:
                        # non-critical DMAs ride behind the critical set:
                        # this group's bf16 enc (needed at group end) and
                        # the group-after-next's fp8 enc.
                        for b in bs:
                            et_bf[b] = load_encbf(b)
                        for b in range((g + 2) * GB,
                                       min((g + 3) * GB, BL)):
                            et_q[b] = load_encq(b)
                    if g == 1 and ft == 8:
                        # wct decoder-half block load (ScalarE queue)
                        wct_dec = consts.tile([128, KH * H], bf16)
                        nc.scalar.dma_start(wct_dec[:], wct[0])
                    if g == 2 and ft == 4:
                        # final combine, decoder_out half: out^T[f,b]
                        # accumulates in one PSUM bank with 8 sub-bank
                        # slices (single-start rule, see ps_pre).
                        psOut_pool = ctx.enter_context(
                            tc.tile_pool(name="psOut", bufs=1, space="PSUM"))
                        ps_out = psOut_pool.tile([128, 8 * BL], f32,
                                                 tag="out", name="psout")
                        for kc in range(KH):
                            for fc in range(8):
                                nc.tensor.matmul(
                                    ps_out[:, fc * BL:(fc + 1) * BL],
                                    wct_dec[:, kc * H + fc * 128:
                                            kc * H + (fc + 1) * 128],
                                    decT_sb[:, kc * BL:(kc + 1) * BL],
                                    start=(kc == 0 and fc == 0), stop=False,
                                    skip_group_check=True)
                    if g == 2 and ft == 8:
                        # wct applied-half block load (ScalarE queue)
                        wct_app = consts.tile([128, KH * H], bf16)
                        nc.scalar.dma_start(wct_app[:], wct[1])

                # last ft's scores, then softmax/apply for this group
                # (the V/S/DMA chain overlaps the next group's PE sweep)
                for (pft, pb) in pending:
                    emit_scores_mm(psc[pb], pft, pb)
                pending = []
                for b in bs:
                    emit_softmax_apply(psc[b], b)

            # ---- final combine, applied half + bias + tanh ----------------
            for kc in range(KH):
                for fc in range(8):
                    nc.tensor.matmul(
                        ps_out[:, fc * BL:(fc + 1) * BL],
                        wct_app[:, kc * H + fc * 128:
                                kc * H + (fc + 1) * 128],
                        appT_bf[:, kc * BL:(kc + 1) * BL],
                        start=False, stop=False,
                        skip_group_check=True)
            for fc in range(8):
                nc.tensor.matmul(
                    ps_out[:, fc * BL:(fc + 1) * BL],
                    b_comb_sb[:, fc * 128:(fc + 1) * 128],
                    ones8[:],
                    start=False, stop=(fc == 7),
                    skip_group_check=True)

            outT_sb = consts.tile([128, 8 * BL], f32)
            nc.scalar.activation(outT_sb[:], ps_out[:], AF.Tanh)
            nc.sync.dma_start(outT_d[:], outT_sb[:])
            nc.sync.dma_start(appT_d[:], appT_sb[:])

    nc.compile()
    return nc


def _get_nc():
    if "nc" not in _CACHE:
        _CACHE["nc"] = _build()
    return _CACHE["nc"]


def make_in_maps(inputs):
    import ml_dtypes
    bf = ml_dtypes.bfloat16
    f8 = ml_dtypes.float8_e4m3fn

    inp = {k: np.asarray(v, dtype=np.float32) for k, v in inputs.items()}
    hidden = inp["hidden"]
    decoder_out = inp["decoder_out"]
    encoder_states = inp["encoder_states"]
    W_attn = inp["W_attn"]
    b_attn = inp["b_attn"]
    W_attn2 = inp["W_attn2"]
    W_comb = inp["W_comb"]
    b_comb = inp["b_comb"]
    # b_attn2 shifts every score equally -> softmax-invariant, unused.

    watT = np.ascontiguousarray(W_attn.T)                     # [2H, 2H]
    w1h = np.ascontiguousarray(watT[:H] * WSCALE).astype(f8)  # hidden-half
    w2t = np.ascontiguousarray(watT[H:] * WSCALE).astype(f8)  # encoder-half
    wct = np.ascontiguousarray(
        W_comb.T.reshape(2, KH, 128, H).transpose(0, 2, 1, 3)
        .reshape(2, 128, KH * H)).astype(bf)
    w2rep = np.repeat(W_attn2.reshape(F, 1), BL, axis=1)
    bias = np.ascontiguousarray(np.concatenate(
        [b_attn.reshape(1, F) * WSCALE, b_comb.reshape(1, H)],
        axis=1).astype(np.float32))

    in_maps = []
    for c in range(NCORES):
        sl = slice(c * BL, (c + 1) * BL)
        enc_pc = np.ascontiguousarray(
            encoder_states[:, sl, :].transpose(1, 2, 0)
            .reshape(BL, KH, 128, S).transpose(0, 2, 1, 3)
            .reshape(BL, 128, KH * S))                        # [BL, p, k*s]
        smalls = np.concatenate(
            [hidden[sl].T, decoder_out[sl].T, w2rep],
            axis=0).astype(np.float32)                        # [4H, BL]
        smalls = np.ascontiguousarray(
            smalls.reshape(32, 128, BL).transpose(1, 0, 2)
            .reshape(128, 32 * BL))                           # [p, k*BL+b]
        in_maps.append({
            "enc_t": enc_pc.astype(bf),
            "enc_q": enc_pc.astype(f8),
            "w1h": w1h,
            "w2t": w2t,
            "wct": wct,
            "smalls": smalls,
            "bias": bias,
        })
    return in_maps


def kernel(**inputs):
    from concourse.bass_utils import run_bass_kernel_spmd

    in_maps = make_in_maps(inputs)
    nc = _get_nc()
    res = run_bass_kernel_spmd(nc, in_maps, list(range(NCORES)))
    def unarr(a):
        # [128, k*BL+b] -> [k*128+p, b] -> [b, kp]
        return a.reshape(128, -1, BL).transpose(1, 0, 2).reshape(H, BL).T

    out = np.concatenate(
        [unarr(res.results[c]["outT"]) for c in range(NCORES)], axis=0)
    applied = np.concatenate(
        [unarr(res.results[c]["appliedT"]) for c in range(NCORES)], axis=0)
    return out.astype(np.float32), applied.astype(np.float32)
